# revision 15
# baseline (speedup 1.0000x reference)
"""BaGuaLLM Trainium2 kernel: 8-core batch-parallel, feature-major layout.

Activations live feature-major in SBUF: [128, KT*256] where global feature
f = ki*128 + p sits at [p, ki*256 + t]. Matmuls contract features on the
partition dim; weights are host-packed tile-major bf16. Cross-core traffic:
one 64B AllReduce per layer (head-elimination stats) plus one for the
scene/causal flag.
"""
import sys
sys.path.insert(0, '/opt/trn_rl_repo')
import numpy as np
import ml_dtypes

BF = ml_dtypes.bfloat16
N_CORES = 8
DIM, KT, S, NH, HD, FF, VOCAB, LFULL = 768, 6, 256, 8, 96, 3072, 32000, 12
HEADMT = VOCAB // 128  # 250
THR = 0.3
SCENE_W = np.array([0.95,0.95,0.9,0.85,0.8,0.95,0.9,0.05,0.05,0.1,0.1,0.05,
                    0.05,0.05,0.1,0.05,0.4,0.7,0.35,0.45,0.6,0.5,0.65],
                   dtype=np.float32)

# per-layer bias blob columns
OUT_B, FFB1, FFB2, LN1G, LN1B, FFLNG, FFLNB, SH1B, SH2B, POLB = \
    0, 6, 30, 36, 42, 48, 54, 60, 63, 64
IMPW1, IMPB1, IMPW2, IMPB2 = 96, 224, 352, 480
NBIAS = 481
# static bias blob columns
ZRWB, ZWWB, ZMTB, ZMPB, ZOGB, HLNG, HLNB, TSB1, TSB2, CGB, DGB = \
    0, 1, 2, 3, 9, 15, 21, 27, 30, 31, 37
EPSC, IGB, NEG1 = 43, 44, 45
NSBIAS = 46


def tilemaj(w):
    K, M = w.shape
    nk = K // 128
    if M % 128:
        w = np.concatenate([w, np.zeros((K, 128 - M % 128), w.dtype)], 1)
    nm = w.shape[1] // 128
    return np.ascontiguousarray(
        w.reshape(nk, 128, nm, 128).transpose(1, 0, 2, 3).reshape(128, nk*nm*128)), nk, nm


def tilemaj_narrow(w):
    K, M = w.shape
    nk = K // 128
    return np.ascontiguousarray(
        w.reshape(nk, 128, M).transpose(1, 0, 2).reshape(128, nk*M))


def ktile(w):
    K, N = w.shape
    nk = K // 128
    return np.ascontiguousarray(
        w.reshape(nk, 128, N).transpose(1, 0, 2).reshape(128, nk*N))


def colpack(v):
    return np.ascontiguousarray(v.reshape(-1, 128).T)


def pack_banded(w, nk, nm):
    slots, smap = [], []
    for mi in range(nm):
        ks = [ki for ki in range(nk)
              if np.abs(w[ki*128:(ki+1)*128, mi*128:(mi+1)*128]).max() > 0]
        ks = ks if ks else [0]
        smap.append([(ki, len(slots) + j) for j, ki in enumerate(ks)])
        slots.extend((ki, mi) for ki in ks)
    blob = np.zeros((128, len(slots)*128), w.dtype)
    for si, (ki, mi) in enumerate(slots):
        blob[:, si*128:(si+1)*128] = w[ki*128:(ki+1)*128, mi*128:(mi+1)*128]
    return blob, smap


def seg_rows(n):
    segs = []
    r0, r1 = n*96, (n+1)*96
    ki = r0 // 128
    while r0 < r1:
        hi = min(r1, (ki+1)*128)
        segs.append((ki, r0 - ki*128, hi - ki*128, r0 - n*96))
        r0 = hi
        ki += 1
    return segs


def prep_host(params, n_layers):
    p = {k: np.asarray(v) for k, v in params.items()}
    g, meta = {}, {}
    g['emb'] = np.ascontiguousarray(p['embedding'].astype(np.float32))

    trig_l, out_l, ff1_l, ff2_l, sh1_l, sh2_l, pol_l, bias_l = ([] for _ in range(8))
    for l in range(n_layers):
        tw = p['trig_w'][l] * np.cos(p['res_freq'][l] * np.pi)[:, None, :]
        TW = tw.transpose(1, 0, 2).reshape(DIM, DIM)
        trig_l.append(tilemaj(TW.astype(BF))[0])
        out_l.append(tilemaj(p['out_w'][l].astype(BF))[0])
        W1 = p['ff_w1'][l].astype(BF)
        ff1_l.append(np.concatenate(
            [tilemaj(W1[:, h*1536:(h+1)*1536])[0] for h in range(2)], 1))
        W2 = p['ff_w2'][l].astype(BF)
        ff2_l.append(np.concatenate(
            [tilemaj(W2[h*1536:(h+1)*1536, :])[0] for h in range(2)], 1))
        S1 = np.zeros((DIM, NH*48), np.float32)
        for n in range(NH):
            S1[n*96:(n+1)*96, n*48:(n+1)*48] = p['sh_w1'][l, n]
        blob1, smap1 = pack_banded(S1.astype(BF), KT, 3)
        sh1_l.append(blob1)
        if l == 0:
            meta['sh1_map'] = smap1
            meta['sh1_slots'] = blob1.shape[1] // 128
        S2 = np.zeros((NH*48, NH), np.float32)
        for n in range(NH):
            S2[n*48:(n+1)*48, n] = p['sh_w2'][l, n, :, 0]
        sh2_l.append(tilemaj_narrow(S2.astype(BF)))
        PS = np.zeros((DIM, 32), np.float32)
        for n in range(NH):
            PS[n*96:(n+1)*96, :] = p['pol_w'][l, n] / S
        pol_l.append(ktile(PS.astype(BF)))
        b = np.zeros((128, NBIAS), np.float32)
        b[:, OUT_B:OUT_B+6] = colpack(p['out_b'][l])
        b[:, FFB1:FFB1+24] = colpack(p['ff_b1'][l])
        b[:, FFB2:FFB2+6] = colpack(p['ff_b2'][l])
        b[:, LN1G:LN1G+6] = colpack(p['ln1_g'][l])
        b[:, LN1B:LN1B+6] = colpack(p['ln1_b'][l])
        b[:, FFLNG:FFLNG+6] = colpack(p['ff_ln_g'][l])
        b[:, FFLNB:FFLNB+6] = colpack(p['ff_ln_b'][l])
        b[:, SH1B:SH1B+3] = np.ascontiguousarray(
            p['sh_b1'][l].reshape(-1).reshape(3, 128).T)
        b[:8, SH2B] = p['sh_b2'][l, :, 0]
        b[:8, POLB:POLB+32] = p['pol_b'][l]
        b[:8, IMPW1:IMPW1+128] = np.tile(p['imp_w1'][l, 0], 8)[None, :].repeat(8, 0)
        b[:8, IMPB1:IMPB1+128] = np.tile(p['imp_b1'][l], 8)[None, :].repeat(8, 0)
        b[:8, IMPW2:IMPW2+128] = np.tile(p['imp_w2'][l, :, 0], 8)[None, :].repeat(8, 0)
        b[:8, IMPB2] = p['imp_b2'][l, 0]
        bias_l.append(b)
    cat = lambda xs: np.ascontiguousarray(np.concatenate(xs, 1))
    g['trig'], g['outw'] = cat(trig_l), cat(out_l)
    g['ff1w'], g['ff2w'] = cat(ff1_l), cat(ff2_l)
    g['sh1w'], g['sh2w'], g['polw'], g['biasb'] = \
        cat(sh1_l), cat(sh2_l), cat(pol_l), cat(bias_l)

    Z = np.zeros((DIM, DIM), np.float32)
    for n in range(NH):
        Z[n*96:(n+1)*96, n*96:(n+1)*96] = p['zone_w'][n, :96, :]
    zb, zmap = pack_banded(Z.astype(BF), KT, KT)
    g['zonew'] = zb
    meta['zone_map'] = zmap
    meta['zone_slots'] = zb.shape[1] // 128
    pos = np.arange(S, dtype=np.float32) / (S - 1)
    zs = S / NH
    zr = (np.arange(S, dtype=np.float32) % zs) / zs
    tc_ = np.arange(NH, dtype=np.float32) / 7.0
    pc = np.stack([pos[None, :]*0.5 + tc_[:, None]*0.5,
                   np.broadcast_to(zr, (NH, S))], -1)
    Z2 = np.zeros((16, DIM), np.float32)
    for n in range(NH):
        Z2[n*2:(n+1)*2, n*96:(n+1)*96] = p['zone_w'][n, 96:98, :]
    g['zw2'] = np.ascontiguousarray(Z2.reshape(16, KT, 128).reshape(16, KT*128)).astype(BF)
    g['pc16'] = np.ascontiguousarray(
        pc.transpose(0, 2, 1).reshape(16, S)).astype(BF)
    G = np.zeros((DIM, NH), np.float32)
    for n in range(NH):
        G[n*96:(n+1)*96, n] = p['ig_w'][:, 0]
    g['igw'] = tilemaj_narrow(G.astype(BF))
    g['cgw'] = tilemaj(p['buf_cg_w'].astype(BF))[0]
    g['dgw'] = tilemaj(p['buf_dg_w'].astype(BF))[0]
    g['posT'] = ktile(np.ascontiguousarray(p['pos_embedding'][:S].T).astype(np.float32))
    g['tsa1'] = tilemaj(p['tsa_w1'].astype(BF))[0]
    g['tsa2'] = tilemaj_narrow(p['tsa_w2'].astype(BF))
    g['zrww'] = tilemaj_narrow(p['z_rw_w'].astype(BF))
    g['zwww'] = tilemaj_narrow(p['z_ww_w'].astype(BF))
    g['zmtw'] = tilemaj_narrow(p['z_mt_w'].astype(BF))
    zmp = np.zeros((128, 6*128), BF)
    zmp[:32] = np.ascontiguousarray(p['z_mp_w']).astype(BF).reshape(32, 768)
    g['zmpw'] = zmp
    g['zogw'] = tilemaj(p['z_og_w'].astype(BF))[0]
    g['headw'] = tilemaj(p['head_w'].astype(BF))[0]
    fr = np.arange(NH*96) // 96
    e8 = (fr[None, :] == np.arange(NH)[:, None]).astype(np.float32)
    g['e8bf'] = e8.astype(BF)
    e8c = np.zeros((128, KT*8), np.float32)
    ist = np.zeros((128, KT*96), np.float32)
    for ki in range(KT):
        rows = np.arange(128) + ki*128
        e8c[:, ki*8:(ki+1)*8] = (rows[:, None]//96 == np.arange(8)[None, :])
        ist[:, ki*96:(ki+1)*96] = ((rows % 96)[:, None] == np.arange(96)[None, :])
    g['e8c'] = e8c
    g['e8cb'] = e8c.astype(BF)
    g['istack'] = ist.astype(BF)
    g['eyec'] = (1.0 - np.eye(8)).astype(np.float32)
    g['pre8'] = (0.1 * (1.0 - np.eye(8))).astype(np.float32)
    scw = np.zeros((1, 32), np.float32)
    scw[0, :23] = SCENE_W
    g['scw'] = scw
    g['causm1'] = np.ascontiguousarray(
        ((np.arange(S, dtype=np.float32)+1.0)/S - 1.0)[None, :])
    g['onesb'] = np.ones((128, 1), BF)
    sb = np.zeros((128, NSBIAS), np.float32)
    sb[:32, ZRWB] = p['z_rw_b']
    sb[:32, ZWWB] = p['z_ww_b']
    sb[:32, ZMTB] = p['z_mt_b']
    sb[:, ZMPB:ZMPB+6] = colpack(p['z_mp_b'])
    sb[:, ZOGB:ZOGB+6] = colpack(p['z_og_b'])
    sb[:, HLNG:HLNG+6] = colpack(p['head_ln_g'])
    sb[:, HLNB:HLNB+6] = colpack(p['head_ln_b'])
    sb[:, TSB1:TSB1+3] = np.ascontiguousarray(p['tsa_b1'].reshape(3, 128).T)
    sb[:23, TSB2] = p['tsa_b2']
    sb[:, CGB:CGB+6] = colpack(p['buf_cg_b'])
    sb[:, DGB:DGB+6] = colpack(p['buf_dg_b'])
    sb[:, EPSC] = 1e-5
    sb[:, IGB] = float(p['ig_b'][0])
    sb[:, NEG1] = -1.0
    g['sbias'] = sb
    meta['ig_b'] = float(p['ig_b'][0])
    return g, meta


def build(n_layers, head_mt, meta, dbg=False):
    from concourse import bass, mybir, tile, bacc
    from concourse.masks import make_identity
    dt = mybir.dt
    F32, BF16, I32 = dt.float32, dt.bfloat16, dt.int32
    AF = mybir.ActivationFunctionType
    OP = mybir.AluOpType
    ig_b = meta['ig_b']

    nc = bacc.Bacc("TRN2", target_bir_lowering=False, debug=False,
                   num_devices=N_CORES)
    D = lambda name, shape, dty: nc.dram_tensor(name, shape, dty,
                                                kind="ExternalInput").ap()
    emb = D("emb", [VOCAB, DIM], F32)
    ids = D("ids", [128, 2], I32)
    trig = D("trig", [128, n_layers*36*128], BF16)
    outw = D("outw", [128, n_layers*36*128], BF16)
    ff1w = D("ff1w", [128, n_layers*144*128], BF16)
    ff2w = D("ff2w", [128, n_layers*144*128], BF16)
    NSH1 = meta["sh1_slots"]
    sh1w = D("sh1w", [128, n_layers*NSH1*128], BF16)
    sh2w = D("sh2w", [128, n_layers*24], BF16)
    polw = D("polw", [128, n_layers*192], BF16)
    biasb = D("biasb", [128, n_layers*NBIAS], F32)
    NZONE = meta["zone_slots"]
    zonew = D("zonew", [128, NZONE*128], BF16)
    zw2 = D("zw2", [16, KT*128], BF16)
    pc16 = D("pc16", [16, S], BF16)
    igw = D("igw", [128, KT*8], BF16)
    cgw = D("cgw", [128, 36*128], BF16)
    dgw = D("dgw", [128, 36*128], BF16)
    posT = D("posT", [128, KT*S], F32)
    tsa1 = D("tsa1", [128, 18*128], BF16)
    tsa2 = D("tsa2", [128, 3*23], BF16)
    zrww = D("zrww", [128, KT*32], BF16)
    zwww = D("zwww", [128, KT*32], BF16)
    zmtw = D("zmtw", [128, KT*32], BF16)
    zmpw = D("zmpw", [128, 6*128], BF16)
    zogw = D("zogw", [128, 72*128], BF16)
    headw = D("headw", [128, KT*HEADMT*128], BF16)
    e8bf = D("e8bf", [8, DIM], BF16)
    e8c = D("e8c", [128, KT*8], F32)
    e8cb = D("e8cb", [128, KT*8], BF16)
    istack = D("istack", [128, KT*96], BF16)
    eyec = D("eyec", [8, 8], F32)
    pre8 = D("pre8", [8, 8], F32)
    scw = D("scw", [1, 32], F32)
    causm1 = D("causm1", [1, S], F32)
    onesb = D("onesb", [128, 1], BF16)
    sbias = D("sbias", [128, NSBIAS], F32)
    out = nc.dram_tensor("out", [128, head_mt*S], F32, kind="ExternalOutput").ap()
    if dbg:
        dbgT = nc.dram_tensor("dbg", [128, 12416], F32, kind="ExternalOutput").ap()
        dbgB = nc.dram_tensor("dbgb", [128, 10752], BF16, kind="ExternalOutput").ap()

    nAR = n_layers + 1
    arin = [nc.dram_tensor(f"ari{i}", [64, 1], F32).ap() for i in range(nAR)]
    arout = [nc.dram_tensor(f"aro{i}", [64, 1], F32, addr_space="Shared").ap()
             for i in range(nAR)]

    with tile.TileContext(nc) as tc:
        import contextlib
        with contextlib.ExitStack() as ctx:
            sp = ctx.enter_context(tc.tile_pool(name="static", bufs=1))
            wp = ctx.enter_context(tc.tile_pool(name="wstream", bufs=1))
            ap_ = ctx.enter_context(tc.tile_pool(name="acts", bufs=1))
            rp = ctx.enter_context(tc.tile_pool(name="rows", bufs=1))
            pp = ctx.enter_context(tc.tile_pool(name="ps", bufs=1, space="PSUM"))

            def T(pool, shape, dty, tag, name, bufs=1):
                return pool.tile(shape, dty, tag=tag, name=name, bufs=bufs)

            def ld(dram, shape, dty, tag, name):
                t = T(sp, shape, dty, tag, name)
                nc.sync.dma_start(out=t[:], in_=dram[:])
                return t

            zone_s = ld(zonew, [128, NZONE*128], BF16, "zone", "zone_s")
            zw2_s = ld(zw2, [16, KT*128], BF16, "zw2S", "zw2_s")
            pc16_s = ld(pc16, [16, S], BF16, "pc16S", "pc16_s")
            igw_s = ld(igw, [128, KT*8], BF16, "igwS", "igw_s")
            cgw_s = ld(cgw, [128, 36*128], BF16, "cgwS", "cgw_s")
            dgw_s = ld(dgw, [128, 36*128], BF16, "dgwS", "dgw_s")
            e8bf_s = ld(e8bf, [8, DIM], BF16, "e8bfS", "e8bf_s")
            e8c_s = ld(e8c, [128, KT*8], F32, "e8cS", "e8c_s")
            e8cb_s = ld(e8cb, [128, KT*8], BF16, "e8cbS", "e8cb_s")
            ist_s = ld(istack, [128, KT*96], BF16, "istS", "ist_s")
            eyec_s = ld(eyec, [8, 8], F32, "eyecS", "eyec_s")
            pre8_s = ld(pre8, [8, 8], F32, "pre8S", "pre8_s")
            scw_s = ld(scw, [1, 32], F32, "scwS", "scw_s")
            causm1_s = ld(causm1, [1, S], F32, "causS", "causm1_s")
            ones_s = ld(onesb, [128, 1], BF16, "onesS", "ones_s")
            sbias_s = ld(sbias, [128, NSBIAS], F32, "sbiasS", "sbias_s")
            ident = T(sp, [128, 128], F32, "identS", "ident_s")
            make_identity(nc, ident[:])
            # scratch 32x32 tiles
            pv8 = T(sp, [32, 32], F32, "pv8S", "pv8_s")
            pvT = T(sp, [32, 32], F32, "pvTS", "pvT_s")
            i32t = T(sp, [32, 32], F32, "i32S", "i32_s")
            it32 = T(sp, [32, 32], F32, "it32S", "it32_s")
            c32 = T(sp, [32, 32], F32, "c32S", "c32_s")
            ct32 = T(sp, [32, 32], F32, "ct32S", "ct32_s")
            z23 = T(sp, [32, 32], F32, "z23S", "z23_s")
            zt = T(sp, [32, 32], F32, "ztS", "zt_s")
            fbc = T(sp, [128, S], F32, "fbcS", "fbc_s")

            # ---- embedding gather -> feature-major x
            ids_sb = T(sp, [128, 2], I32, "idsS", "ids_sb")
            nc.sync.dma_start(out=ids_sb[:], in_=ids[:])
            x = T(ap_, [128, KT*S], F32, "x0", "x_l0")
            for hi in range(2):
                gx = T(ap_, [128, DIM], F32, "gx", f"gx{hi}")
                nc.gpsimd.indirect_dma_start(
                    out=gx[:], out_offset=None, in_=emb[:],
                    in_offset=bass.IndirectOffsetOnAxis(ap=ids_sb[:, hi:hi+1], axis=0))
                for fi in range(KT):
                    pt = pp.tile([128, S], F32, tag=f"pm{fi}", name=f"tp{hi}_{fi}")
                    nc.tensor.transpose(out=pt[:, :128], in_=gx[:, fi*128:(fi+1)*128],
                                        identity=ident[:])
                    nc.scalar.copy(out=x[:, fi*S+hi*128:fi*S+hi*128+128],
                                   in_=pt[:, :128])
            pos_s = T(wp, [128, 36*128], F32, "outw", "pos_s")
            nc.sync.dma_start(out=pos_s[:, :KT*S], in_=posT[:])
            for ki in range(KT):
                nc.vector.tensor_tensor(out=x[:, ki*S:(ki+1)*S], in0=x[:, ki*S:(ki+1)*S],
                                        in1=pos_s[:, ki*S:(ki+1)*S], op=OP.add)
            x_bf = T(ap_, [128, KT*S], BF16, "xbf0", "xbf_l0")
            for ki in range(KT):
                nc.scalar.copy(out=x_bf[:, ki*S:(ki+1)*S], in_=x[:, ki*S:(ki+1)*S])
            if dbg:
                nc.sync.dma_start(out=dbgT[:, 0:1536], in_=x[:])

            # ---- scene -> use_causal -> factor broadcast
            tsa1_s = T(wp, [128, 36*128], BF16, "trigw", "tsa1_s")
            nc.sync.dma_start(out=tsa1_s[:, :18*128], in_=tsa1[:])
            tsa2_s = T(sp, [128, 3*23], BF16, "tsa2S", "tsa2_s")
            nc.sync.dma_start(out=tsa2_s[:], in_=tsa2[:])
            tg = T(sp, [128, 3], BF16, "tgS", "tg_s")
            for mi in range(3):
                pt = pp.tile([128, 1], F32, tag="pr0", name=f"tsa_{mi}")
                for ki in range(KT):
                    nc.tensor.matmul(pt[:], tsa1_s[:, (ki*3+mi)*128:(ki*3+mi+1)*128],
                                     x_bf[:, ki*S:ki*S+1],
                                     start=(ki == 0), stop=(ki == KT-1))
                nc.scalar.activation(tg[:, mi:mi+1], pt[:], AF.Gelu,
                                     bias=sbias_s[:, TSB1+mi:TSB1+mi+1])
            nc.gpsimd.memset(z23[:], 0.0)
            pz = pp.tile([23, 1], F32, tag="pr0", name="pz23")
            for ki in range(3):
                nc.tensor.matmul(pz[:], tsa2_s[:, ki*23:(ki+1)*23], tg[:, ki:ki+1],
                                 start=(ki == 0), stop=(ki == 2))
            nc.vector.tensor_scalar(out=z23[0:23, 0:1], in0=pz[:],
                                    scalar1=sbias_s[0:23, TSB2:TSB2+1], scalar2=None,
                                    op0=OP.add)
            nc.vector.transpose(zt[:], z23[:])
            nc.gpsimd.memset(zt[0:1, 23:32], -1e30)
            mx = T(rp, [1, 1], F32, "r_mx", "sc_mx")
            nc.vector.tensor_reduce(mx[:], zt[0:1, :], axis=mybir.AxisListType.X, op=OP.max)
            exr = T(rp, [1, 32], F32, "r_ex", "sc_ex")
            nc.vector.tensor_scalar(out=exr[:], in0=zt[0:1, :], scalar1=mx[:, :],
                                    scalar2=None, op0=OP.subtract)
            nc.scalar.activation(exr[:], exr[:], AF.Exp)
            smr = T(rp, [1, 1], F32, "r_sm", "sc_sm")
            nc.vector.tensor_reduce(smr[:], exr[:], axis=mybir.AxisListType.X, op=OP.add)
            rsr = T(rp, [1, 1], F32, "r_rs", "sc_rs")
            nc.vector.reciprocal(rsr[:], smr[:])
            nc.vector.tensor_scalar(out=exr[:], in0=exr[:], scalar1=rsr[:, :],
                                    scalar2=None, op0=OP.mult)
            nc.vector.tensor_tensor(out=exr[:], in0=exr[:], in1=scw_s[:], op=OP.mult)
            ucp = T(rp, [1, 1], F32, "r_ucp", "sc_ucp")
            nc.vector.tensor_reduce(ucp[:], exr[:], axis=mybir.AxisListType.X, op=OP.add)
            nc.sync.dma_start(out=arin[n_layers][0:1, :], in_=ucp[:])
            nc.gpsimd.collective_compute(
                "AllReduce", OP.add, ins=[arin[n_layers][:]], outs=[arout[n_layers][:]],
                replica_groups=[list(range(N_CORES))])
            ucs = T(rp, [1, 1], F32, "r_ucs", "sc_ucs")
            nc.sync.dma_start(out=ucs[:], in_=arout[n_layers][0:1, :])
            ucf = T(rp, [1, 1], F32, "r_ucf", "sc_ucf")
            nc.vector.tensor_scalar(out=ucf[:], in0=ucs[:], scalar1=4.0, scalar2=None,
                                    op0=OP.is_gt)
            frow = T(rp, [1, S], F32, "r_frow", "sc_frow")
            nc.vector.tensor_scalar(out=frow[:], in0=causm1_s[:], scalar1=ucf[:, :],
                                    scalar2=1.0, op0=OP.mult, op1=OP.add)
            nc.gpsimd.partition_broadcast(fbc[:], frow[:])

            def mm_block(psum, blob, nm, mi, rhs, kis):
                for j, ki in enumerate(kis):
                    nc.tensor.matmul(psum, blob[:, (ki*nm+mi)*128:(ki*nm+mi+1)*128],
                                     rhs[:, ki*S:(ki+1)*S],
                                     start=(j == 0), stop=(j == len(kis)-1))

            def layernorm(lname, src, gcol, bcol, out_t):
                srcb = T(ap_, [128, KT*S], BF16, "lnb", f"{lname}_srcb")
                sqb = T(ap_, [128, KT*S], BF16, "lnsq", f"{lname}_sqb")
                for ki in range(KT):
                    nc.scalar.copy(out=srcb[:, ki*S:(ki+1)*S], in_=src[:, ki*S:(ki+1)*S])
                    nc.scalar.activation(sqb[:, ki*S:(ki+1)*S], srcb[:, ki*S:(ki+1)*S],
                                         AF.Square)
                pA = pp.tile([1, S], F32, tag="pr0", name=f"{lname}_pA")
                pB = pp.tile([1, S], F32, tag="pr1", name=f"{lname}_pB")
                for ki in range(KT):
                    nc.tensor.matmul(pA[:], ones_s[:, 0:1], srcb[:, ki*S:(ki+1)*S],
                                     start=(ki == 0), stop=(ki == KT-1))
                for ki in range(KT):
                    nc.tensor.matmul(pB[:], ones_s[:, 0:1], sqb[:, ki*S:(ki+1)*S],
                                     start=(ki == 0), stop=(ki == KT-1))
                m = T(rp, [1, S], F32, "r_lm", f"{lname}_m")
                e2 = T(rp, [1, S], F32, "r_le", f"{lname}_e2")
                nc.scalar.activation(m[:], pA[:], AF.Copy, scale=1.0/DIM)
                nc.scalar.activation(e2[:], pB[:], AF.Copy, scale=1.0/DIM)
                var = T(rp, [1, S], F32, "r_lv", f"{lname}_var")
                nc.vector.tensor_tensor(out=var[:], in0=m[:], in1=m[:], op=OP.mult)
                nc.vector.tensor_tensor(out=var[:], in0=e2[:], in1=var[:], op=OP.subtract)
                sd = T(rp, [1, S], F32, "r_le", f"{lname}_sd")
                nc.scalar.activation(sd[:], var[:], AF.Sqrt,
                                     bias=sbias_s[0:1, EPSC:EPSC+1])
                arow = T(rp, [1, S], F32, "r_la", f"{lname}_ar")
                nc.vector.reciprocal(arow[:], sd[:])
                brow = T(rp, [1, S], F32, "r_lv", f"{lname}_br")
                nc.vector.tensor_tensor(out=brow[:], in0=m[:], in1=arow[:], op=OP.mult)
                nc.vector.tensor_scalar(out=brow[:], in0=brow[:], scalar1=-1.0,
                                        scalar2=None, op0=OP.mult)
                aB = T(ap_, [128, S], F32, "aB", f"{lname}_aB")
                bB = T(ap_, [128, S], F32, "bB", f"{lname}_bB")
                nc.gpsimd.partition_broadcast(aB[:], arow[:])
                nc.gpsimd.partition_broadcast(bB[:], brow[:])
                for ki in range(KT):
                    t1 = T(ap_, [128, S], F32, "lnt", f"{lname}_t{ki}", bufs=2)
                    nc.vector.tensor_tensor(out=t1[:], in0=src[:, ki*S:(ki+1)*S],
                                            in1=aB[:], op=OP.mult)
                    nc.vector.tensor_tensor(out=t1[:], in0=t1[:], in1=bB[:], op=OP.add)
                    nc.vector.tensor_scalar(out=out_t[:, ki*S:(ki+1)*S], in0=t1[:],
                                            scalar1=gcol[:, ki:ki+1],
                                            scalar2=bcol[:, ki:ki+1],
                                            op0=OP.mult, op1=OP.add)

            # ================= layers =================
            for l in range(n_layers):
                trig_w = T(wp, [128, 36*128], BF16, "trigw", f"trig_{l}")
                nc.sync.dma_start(out=trig_w[:], in_=trig[:, l*36*128:(l+1)*36*128])
                out_w = T(wp, [128, 36*128], BF16, "outw", f"outw_{l}")
                nc.sync.dma_start(out=out_w[:], in_=outw[:, l*36*128:(l+1)*36*128])
                sh1_w = T(wp, [128, NSH1*128], BF16, "sh1w", f"sh1w_{l}")
                nc.sync.dma_start(out=sh1_w[:],
                                  in_=sh1w[:, l*NSH1*128:(l+1)*NSH1*128])
                sh2_w = T(wp, [128, 24], BF16, "sh2w", f"sh2w_{l}")
                nc.sync.dma_start(out=sh2_w[:], in_=sh2w[:, l*24:(l+1)*24])
                pol_w = T(wp, [128, 192], BF16, "polw", f"polw_{l}")
                nc.sync.dma_start(out=pol_w[:], in_=polw[:, l*192:(l+1)*192])
                bias = T(wp, [128, NBIAS], F32, "biasb", f"bias_{l}")
                nc.sync.dma_start(out=bias[:], in_=biasb[:, l*NBIAS:(l+1)*NBIAS])

                heads_bf = T(ap_, [128, KT*S], BF16, "heads", f"heads_{l}")
                hf_bf = T(ap_, [128, KT*S], BF16, "hf", f"hf_{l}")
                hm = T(ap_, [128, KT], F32, "hm", f"hm_{l}")
                for mi in range(KT):
                    ps = pp.tile([128, S], F32, tag=f"pm{mi}", name=f"trig_{l}_{mi}")
                    mm_block(ps[:], trig_w, KT, mi, x_bf, range(KT))
                    nc.scalar.copy(out=heads_bf[:, mi*S:(mi+1)*S], in_=ps[:])
                    nc.vector.tensor_tensor(out=hf_bf[:, mi*S:(mi+1)*S],
                                            in0=heads_bf[:, mi*S:(mi+1)*S], in1=fbc[:],
                                            op=OP.mult)
                    nc.vector.tensor_reduce(hm[:, mi:mi+1], heads_bf[:, mi*S:(mi+1)*S],
                                            axis=mybir.AxisListType.X, op=OP.add)
                if dbg and l == 0:
                    nc.sync.dma_start(out=dbgB[:, 0:1536], in_=heads_bf[:])
                    nc.sync.dma_start(out=dbgB[:, 1536:3072], in_=hf_bf[:])
                    nc.sync.dma_start(out=dbgT[:, 12345:12351], in_=hm[:])
                polL = T(ap_, [128, KT*8], BF16, "polL", f"polL_{l}")
                for ki in range(KT):
                    nc.vector.tensor_scalar(out=polL[:, ki*8:(ki+1)*8],
                                            in0=e8cb_s[:, ki*8:(ki+1)*8],
                                            scalar1=hm[:, ki:ki+1], scalar2=None,
                                            op0=OP.mult)
                ppv = pp.tile([8, 32], F32, tag="pr0", name=f"pv_{l}")
                for ki in range(KT):
                    nc.tensor.matmul(ppv[:], polL[:, ki*8:(ki+1)*8],
                                     pol_w[:, ki*32:(ki+1)*32],
                                     start=(ki == 0), stop=(ki == KT-1))
                nc.gpsimd.memset(pv8[:], 0.0)
                nc.vector.tensor_tensor(out=pv8[0:8, :], in0=ppv[:],
                                        in1=bias[0:8, POLB:POLB+32], op=OP.add)
                nc.scalar.activation(pv8[0:8, :], pv8[0:8, :], AF.Tanh)
                if dbg and l == 0:
                    nc.sync.dma_start(out=dbgT[0:32, 12352:12384], in_=pv8[:])
                sq8 = T(rp, [8, 32], F32, "r_sq8", f"sq8_{l}")
                nc.vector.tensor_tensor(out=sq8[:], in0=pv8[0:8, :], in1=pv8[0:8, :],
                                        op=OP.mult)
                dnm = T(rp, [8, 1], F32, "r_dnm", f"dnm_{l}")
                nc.vector.tensor_reduce(dnm[:], sq8[:], axis=mybir.AxisListType.X, op=OP.add)
                nc.scalar.activation(dnm[:], dnm[:], AF.Sqrt)
                nc.vector.tensor_scalar(out=dnm[:], in0=dnm[:], scalar1=1e-12,
                                        scalar2=None, op0=OP.max)
                inv = T(rp, [8, 1], F32, "r_inv", f"inv_{l}")
                nc.vector.reciprocal(inv[:], dnm[:])
                nc.vector.transpose(pvT[:], pv8[:])
                pG = pp.tile([8, 8], F32, tag="pr1", name=f"G_{l}")
                nc.tensor.matmul(pG[:], pvT[:, 0:8], pvT[:, 0:8], start=True, stop=True)
                dpa = T(rp, [8, 8], F32, "r_dpa", f"dpa_{l}")
                nc.vector.tensor_scalar(out=dpa[:], in0=pG[:], scalar1=inv[:, :],
                                        scalar2=None, op0=OP.mult)
                nc.gpsimd.memset(i32t[:], 0.0)
                nc.vector.tensor_copy(out=i32t[0:8, 0:1], in_=inv[:])
                nc.vector.transpose(it32[:], i32t[:])
                invb = T(rp, [8, 8], F32, "r_invb", f"invb_{l}")
                nc.gpsimd.partition_broadcast(invb[:], it32[0:1, 0:8])
                dp = T(rp, [8, 8], F32, "r_dp", f"dp_{l}")
                nc.vector.tensor_tensor(out=dp[:], in0=dpa[:], in1=invb[:], op=OP.mult)
                if dbg and l == 0:
                    nc.sync.dma_start(out=dbgT[0:8, 12384:12392], in_=dp[:])
                h1 = T(rp, [8, 128], F32, "r_h1", f"h1_{l}")
                dpb = dp[:, :].rearrange("p (f o) -> p f o", o=1).to_broadcast([8, 8, 16])
                nc.vector.tensor_tensor(out=h1[:], in0=dpb,
                                        in1=bias[0:8, IMPW1:IMPW1+128], op=OP.mult)
                nc.vector.tensor_tensor(out=h1[:], in0=h1[:],
                                        in1=bias[0:8, IMPB1:IMPB1+128], op=OP.add)
                nc.scalar.activation(h1[:], h1[:], AF.Gelu)
                nc.vector.tensor_tensor(out=h1[:], in0=h1[:],
                                        in1=bias[0:8, IMPW2:IMPW2+128], op=OP.mult)
                impv = T(rp, [8, 8], F32, "r_imp", f"impv_{l}")
                nc.vector.tensor_reduce(impv[:],
                                        h1[:, :].rearrange("p (a b) -> p a b", b=16),
                                        axis=mybir.AxisListType.X, op=OP.add)
                nc.vector.tensor_scalar(out=impv[:], in0=impv[:],
                                        scalar1=bias[0:8, IMPB2:IMPB2+1], scalar2=None,
                                        op0=OP.add)
                # softplus(z) = ln(1+e^z) via Newton: w <- w - 1 + y*exp(-w)
                yv = T(rp, [8, 8], F32, "r_spy", f"spy_{l}")
                nc.scalar.activation(yv[:], impv[:], AF.Exp)
                nc.vector.tensor_scalar(out=yv[:], in0=yv[:], scalar1=1.0,
                                        scalar2=None, op0=OP.add)
                sq_ = T(rp, [8, 8], F32, "r_sps", f"sps_{l}")
                nc.scalar.activation(sq_[:], yv[:], AF.Sqrt)
                nc.vector.reciprocal(sq_[:], sq_[:])
                wv = T(rp, [8, 8], F32, "r_spw", f"spw_{l}")
                nc.vector.tensor_scalar(out=wv[:], in0=yv[:], scalar1=-1.0,
                                        scalar2=None, op0=OP.add)
                nc.vector.tensor_tensor(out=wv[:], in0=wv[:], in1=sq_[:], op=OP.mult)
                ev = T(rp, [8, 8], F32, "r_spe", f"spe_{l}")
                for it_ in range(3):
                    nc.scalar.activation(ev[:], wv[:], AF.Exp, scale=-1.0)
                    nc.vector.tensor_tensor(out=ev[:], in0=yv[:], in1=ev[:], op=OP.mult)
                    nc.vector.tensor_tensor(out=wv[:], in0=wv[:], in1=ev[:], op=OP.add)
                    nc.vector.tensor_scalar(out=wv[:], in0=wv[:], scalar1=-1.0,
                                            scalar2=None, op0=OP.add)
                nc.vector.tensor_copy(out=impv[:], in_=wv[:])
                nc.vector.tensor_tensor(out=impv[:], in0=impv[:], in1=eyec_s[:], op=OP.mult)
                nc.vector.tensor_scalar(out=impv[:], in0=impv[:], scalar1=1.0,
                                        scalar2=None, op0=OP.add)
                nc.vector.reciprocal(impv[:], impv[:])
                if dbg and l == 0:
                    nc.sync.dma_start(out=dbgT[0:8, 12400:12408], in_=impv[:])
                coef = T(rp, [8, 8], F32, "r_coef", f"coef_{l}")
                nc.vector.tensor_tensor(out=coef[:], in0=impv[:], in1=pre8_s[:], op=OP.mult)
                if dbg and l == 0:
                    nc.sync.dma_start(out=dbgT[0:8, 12392:12400], in_=coef[:])
                nc.gpsimd.memset(c32[:], 0.0)
                nc.vector.tensor_copy(out=c32[0:8, 0:8], in_=coef[:])
                nc.vector.transpose(ct32[:], c32[:])
                ctb = T(rp, [8, 8], BF16, "r_ctb", f"ctb_{l}")
                nc.scalar.copy(out=ctb[:], in_=ct32[0:8, 0:8])
                c768 = T(ap_, [128, KT*8], F32, "c768", f"c768_{l}")
                for mi in range(KT):
                    pe = pp.tile([128, 8], F32, tag="pr0", name=f"c768_{l}_{mi}")
                    nc.tensor.matmul(pe[:], e8bf_s[:, mi*128:(mi+1)*128], ctb[:],
                                     start=True, stop=True)
                    nc.scalar.copy(out=c768[:, mi*8:(mi+1)*8], in_=pe[:])
                if dbg and l == 0:
                    nc.sync.dma_start(out=dbgT[:, 12288:12336], in_=c768[:])
                merged_bf = T(ap_, [128, KT*S], BF16, "merged", f"merged_{l}")
                # head-ranges per output tile b: (n, c0, c1, h0)
                for b in range(KT):
                    mwb = T(ap_, [128, KT*128], BF16, f"mwB{b % 2}", f"mwb_{l}_{b}")
                    n0, n1 = (128*b)//96, (128*b+127)//96
                    for a in range(KT):
                        for n in range(n0, n1+1):
                            c0 = max(0, n*96 - 128*b)
                            c1 = min(128, (n+1)*96 - 128*b)
                            h0 = 128*b + c0 - n*96
                            eng = nc.vector if (a + n) % 2 == 0 else nc.gpsimd
                            eng.tensor_scalar(
                                out=mwb[:, a*128+c0:a*128+c1],
                                in0=ist_s[:, a*96+h0:a*96+h0+(c1-c0)],
                                scalar1=c768[:, a*8+n:a*8+n+1],
                                scalar2=None, op0=OP.mult)
                    pT = pp.tile([128, S], F32, tag=f"pm{b % 4}", name=f"mix_{l}_{b}")
                    for a in range(KT):
                        nc.tensor.matmul(pT[:], mwb[:, a*128:(a+1)*128],
                                         hf_bf[:, a*S:(a+1)*S],
                                         start=(a == 0), stop=(a == KT-1))
                    nc.vector.tensor_tensor(out=merged_bf[:, b*S:(b+1)*S],
                                            in0=heads_bf[:, b*S:(b+1)*S],
                                            in1=pT[:], op=OP.add)
                if dbg and l == 0:
                    nc.sync.dma_start(out=dbgB[:, 3072:4608], in_=merged_bf[:])
                pre = T(ap_, [128, KT*S], F32, "pre", f"pre_{l}")
                for mi in range(KT):
                    ps = pp.tile([128, S], F32, tag=f"pm{mi}", name=f"outw_{l}_{mi}")
                    mm_block(ps[:], out_w, KT, mi, merged_bf, range(KT))
                    nc.vector.tensor_scalar(out=pre[:, mi*S:(mi+1)*S], in0=ps[:],
                                            scalar1=bias[:, OUT_B+mi:OUT_B+mi+1],
                                            scalar2=None, op0=OP.add)
                    nc.vector.tensor_tensor(out=pre[:, mi*S:(mi+1)*S],
                                            in0=pre[:, mi*S:(mi+1)*S],
                                            in1=x[:, mi*S:(mi+1)*S], op=OP.add)
                if dbg and l == 0:
                    nc.sync.dma_start(out=dbgT[:, 1536:3072], in_=pre[:])
                oh = T(ap_, [128, KT*S], F32, "oh", f"oh_{l}")
                layernorm(f"ln1_{l}", pre, bias[:, LN1G:LN1G+6], bias[:, LN1B:LN1B+6], oh)
                oh_bf = T(ap_, [128, KT*S], BF16, "ohbf", f"ohbf_{l}")
                for ki in range(KT):
                    nc.scalar.copy(out=oh_bf[:, ki*S:(ki+1)*S], in_=oh[:, ki*S:(ki+1)*S])
                if dbg and l == 0:
                    nc.sync.dma_start(out=dbgT[:, 3072:4608], in_=oh[:])
                pg8 = pp.tile([8, S], F32, tag="pr0", name=f"g8_{l}")
                for ki in range(KT):
                    nc.tensor.matmul(pg8[:], igw_s[:, ki*8:(ki+1)*8],
                                     oh_bf[:, ki*S:(ki+1)*S],
                                     start=(ki == 0), stop=(ki == KT-1))
                g8b = T(rp, [8, S], BF16, "r_g8b", f"g8b_{l}")
                nc.scalar.activation(g8b[:], pg8[:], AF.Sigmoid,
                                     bias=sbias_s[0:8, IGB:IGB+1])
                jz = T(ap_, [128, KT*S], F32, "jz", f"jz_{l}")
                jz_bf = T(ap_, [128, KT*S], BF16, "jzbf", f"jzbf_{l}")
                for mi in range(KT):
                    phe = pp.tile([128, S], F32, tag=f"pm{mi}", name=f"he_{l}_{mi}")
                    for (ki, si) in meta['zone_map'][mi]:
                        nc.tensor.matmul(phe[:], zone_s[:, si*128:(si+1)*128],
                                         oh_bf[:, ki*S:(ki+1)*S],
                                         start=(si == meta['zone_map'][mi][0][1]),
                                         stop=False)
                    nc.tensor.matmul(phe[:], zw2_s[:, mi*128:(mi+1)*128], pc16_s[:],
                                     start=False, stop=True)
                    pgx = pp.tile([128, S], F32, tag="pr1", name=f"gx_{l}_{mi}")
                    nc.tensor.matmul(pgx[:], e8bf_s[:, mi*128:(mi+1)*128], g8b[:],
                                     start=True, stop=True)
                    t1 = T(ap_, [128, S], F32, "zt1", f"zt1_{l}_{mi}", bufs=2)
                    nc.vector.tensor_tensor(out=t1[:], in0=phe[:],
                                            in1=oh[:, mi*S:(mi+1)*S], op=OP.subtract)
                    t2 = T(ap_, [128, S], F32, "zt2", f"zt2_{l}_{mi}", bufs=2)
                    nc.vector.tensor_tensor(out=t2[:], in0=t1[:], in1=pgx[:], op=OP.mult)
                    nc.vector.tensor_tensor(out=jz[:, mi*S:(mi+1)*S],
                                            in0=oh[:, mi*S:(mi+1)*S], in1=t2[:], op=OP.add)
                    nc.scalar.copy(out=jz_bf[:, mi*S:(mi+1)*S], in_=jz[:, mi*S:(mi+1)*S])
                if dbg and l == 0:
                    nc.sync.dma_start(out=dbgT[:, 4608:6144], in_=jz[:])
                s1b = T(ap_, [128, 3*S], BF16, "s1b", f"s1b_{l}")
                for mi in range(3):
                    ps1 = pp.tile([128, S], F32, tag=f"pm{mi}", name=f"s1_{l}_{mi}")
                    sl = meta['sh1_map'][mi]
                    for j, (ki, si) in enumerate(sl):
                        nc.tensor.matmul(ps1[:], sh1_w[:, si*128:(si+1)*128],
                                         jz_bf[:, ki*S:(ki+1)*S],
                                         start=(j == 0), stop=(j == len(sl)-1))
                    nc.scalar.activation(s1b[:, mi*S:(mi+1)*S], ps1[:], AF.Gelu,
                                         bias=bias[:, SH1B+mi:SH1B+mi+1])
                pld = pp.tile([8, S], F32, tag="pr0", name=f"ld_{l}")
                for ki in range(3):
                    nc.tensor.matmul(pld[:], sh2_w[:, ki*8:(ki+1)*8],
                                     s1b[:, ki*S:(ki+1)*S],
                                     start=(ki == 0), stop=(ki == 2))
                sig_d = T(rp, [8, S], BF16, "r_sigd", f"sig_{l}")
                lds = T(rp, [8, 1], F32, "r_lds", f"lds_{l}")
                nc.scalar.activation(sig_d[:], pld[:], AF.Sigmoid,
                                     bias=bias[0:8, SH2B:SH2B+1], accum_out=lds[:])
                absc = T(ap_, [128, KT], F32, "absc", f"absc_{l}")
                for ki in range(KT):
                    d6 = T(ap_, [128, S], F32, "zt1", f"d6_{l}_{ki}", bufs=2)
                    nc.vector.tensor_tensor(out=d6[:], in0=jz[:, ki*S:(ki+1)*S],
                                            in1=x[:, ki*S:(ki+1)*S], op=OP.subtract)
                    nc.vector.tensor_reduce(absc[:, ki:ki+1], d6[:],
                                            axis=mybir.AxisListType.X, op=OP.add,
                                            apply_absolute_value=True)
                pcd = pp.tile([8, 1], F32, tag="pr1", name=f"cd_{l}")
                for ki in range(KT):
                    nc.tensor.matmul(pcd[:], e8c_s[:, ki*8:(ki+1)*8], absc[:, ki:ki+1],
                                     start=(ki == 0), stop=(ki == KT-1))
                cds = T(rp, [8, 1], F32, "r_cds", f"cds_{l}")
                nc.vector.tensor_copy(out=cds[:], in_=pcd[:])
                nc.sync.dma_start(out=arin[l][0:8, :], in_=lds[:])
                nc.sync.dma_start(out=arin[l][32:40, :], in_=cds[:])
                nc.gpsimd.collective_compute(
                    "AllReduce", OP.add, ins=[arin[l][:]], outs=[arout[l][:]],
                    replica_groups=[list(range(N_CORES))])
                arb = T(rp, [64, 1], F32, "r_arb", f"arb_{l}")
                nc.sync.dma_start(out=arb[:], in_=arout[l][:])
                if dbg and l == 0:
                    nc.sync.dma_start(out=dbgT[0:8, 12342:12343], in_=lds[:])
                    nc.sync.dma_start(out=dbgT[0:8, 12343:12344], in_=cds[:])
                    nc.sync.dma_start(out=dbgT[0:64, 12344:12345], in_=arb[:])
                ldm = T(rp, [8, 1], F32, "r_ldm", f"ldm_{l}")
                nc.scalar.activation(ldm[:], arb[0:8, :], AF.Copy, scale=1.0/(8*S))
                sgd = T(rp, [8, 1], F32, "r_sgd", f"sgd_{l}")
                nc.scalar.activation(sgd[:], arb[32:40, :], AF.Sigmoid,
                                     scale=10.0/(8*S*HD),
                                     bias=sbias_s[0:8, NEG1:NEG1+1])
                nc.vector.tensor_tensor(out=ldm[:], in0=ldm[:], in1=sgd[:], op=OP.mult)
                mask8 = T(rp, [8, 1], F32, "r_msk", f"mask8_{l}")
                nc.vector.tensor_scalar(out=mask8[:], in0=ldm[:], scalar1=THR,
                                        scalar2=None, op0=OP.is_ge)
                mask8b = T(rp, [8, 1], BF16, "r_mskb", f"mask8b_{l}")
                nc.scalar.copy(out=mask8b[:], in_=mask8[:])
                m768 = T(ap_, [128, KT], F32, "m768", f"m768_{l}")
                for mi in range(KT):
                    pmx = pp.tile([128, 1], F32, tag="pr0", name=f"mx_{l}_{mi}")
                    nc.tensor.matmul(pmx[:], e8bf_s[:, mi*128:(mi+1)*128], mask8b[:],
                                     start=True, stop=True)
                    nc.scalar.copy(out=m768[:, mi:mi+1], in_=pmx[:])
                if dbg and l == 0:
                    nc.sync.dma_start(out=dbgT[:, 12336:12342], in_=m768[:])
                jzm = T(ap_, [128, KT*S], F32, "pre", f"jzm_{l}")
                jzm_bf = T(ap_, [128, KT*S], BF16, "jzbf2", f"jzmbf_{l}")
                for ki in range(KT):
                    nc.vector.tensor_scalar(out=jzm[:, ki*S:(ki+1)*S],
                                            in0=jz[:, ki*S:(ki+1)*S],
                                            scalar1=m768[:, ki:ki+1], scalar2=None,
                                            op0=OP.mult)
                    nc.vector.tensor_scalar(out=jzm_bf[:, ki*S:(ki+1)*S],
                                            in0=jz[:, ki*S:(ki+1)*S],
                                            scalar1=m768[:, ki:ki+1], scalar2=None,
                                            op0=OP.mult)
                if dbg and l == 0:
                    nc.sync.dma_start(out=dbgT[:, 6144:7680], in_=jzm[:])
                sm = T(ap_, [128, KT*S], F32, "smx", f"sm_{l}")
                for mi in range(KT):
                    pcg = pp.tile([128, S], F32, tag=f"pm{mi % 3}", name=f"cg_{l}_{mi}")
                    mm_block(pcg[:], cgw_s, KT, mi, x_bf, range(KT))
                    cgf = T(ap_, [128, S], F32, "cgf", f"cgf_{l}_{mi}", bufs=2)
                    nc.scalar.activation(cgf[:], pcg[:], AF.Sigmoid,
                                         bias=sbias_s[:, CGB+mi:CGB+mi+1])
                    pdg = pp.tile([128, S], F32, tag=f"pm{3 + mi % 3}", name=f"dg_{l}_{mi}")
                    mm_block(pdg[:], dgw_s, KT, mi, jzm_bf, range(KT))
                    dgf = T(ap_, [128, S], F32, "dgf", f"dgf_{l}_{mi}", bufs=2)
                    nc.scalar.activation(dgf[:], pdg[:], AF.Sigmoid,
                                         bias=sbias_s[:, DGB+mi:DGB+mi+1])
                    t1 = T(ap_, [128, S], F32, "zt1", f"bt1_{l}_{mi}", bufs=2)
                    nc.vector.tensor_tensor(out=t1[:], in0=cgf[:],
                                            in1=x[:, mi*S:(mi+1)*S], op=OP.mult)
                    nc.vector.tensor_tensor(out=t1[:], in0=t1[:], in1=dgf[:], op=OP.mult)
                    nc.vector.tensor_tensor(out=sm[:, mi*S:(mi+1)*S],
                                            in0=jzm[:, mi*S:(mi+1)*S], in1=t1[:],
                                            op=OP.add)
                if dbg and l == 0:
                    nc.sync.dma_start(out=dbgT[:, 7680:9216], in_=sm[:])
                f_bf = T(ap_, [128, KT*S], BF16, "fbf", f"fbf_{l}")
                layernorm(f"ffln_{l}", sm, bias[:, FFLNG:FFLNG+6],
                          bias[:, FFLNB:FFLNB+6], f_bf)
                if dbg and l == 0:
                    nc.sync.dma_start(out=dbgB[:, 4608:6144], in_=f_bf[:])
                pf2s = [pp.tile([128, S], F32, tag=f"pm{mi}", name=f"ff2_{l}_{mi}")
                        for mi in range(KT)]
                for half in range(2):
                    ff1_w = T(wp, [128, 72*128], BF16, "ff1w", f"ff1w_{l}_{half}")
                    nc.sync.dma_start(
                        out=ff1_w[:],
                        in_=ff1w[:, (l*144+half*72)*128:(l*144+(half+1)*72)*128])
                    ff2_w = T(wp, [128, 72*128], BF16, "ff2w", f"ff2w_{l}_{half}")
                    nc.sync.dma_start(
                        out=ff2_w[:],
                        in_=ff2w[:, (l*144+half*72)*128:(l*144+(half+1)*72)*128])
                    h_bf = T(ap_, [128, 12*S], BF16, "hbf", f"hbf_{l}_{half}")
                    for mj in range(12):
                        mi = half*12 + mj
                        ph = pp.tile([128, S], F32, tag=f"pr{mj % 2}",
                                     name=f"ff1_{l}_{mi}")
                        # ff1 half blob: tiles (ki, mi_local=mi-half*12), nm=12
                        for ki in range(KT):
                            nc.tensor.matmul(ph[:],
                                             ff1_w[:, (ki*12+mj)*128:(ki*12+mj+1)*128],
                                             f_bf[:, ki*S:(ki+1)*S],
                                             start=(ki == 0), stop=(ki == KT-1))
                        nc.scalar.activation(h_bf[:, mj*S:(mj+1)*S], ph[:], AF.Gelu,
                                             bias=bias[:, FFB1+mi:FFB1+mi+1])
                    for mi in range(KT):
                        for kj in range(12):
                            # ff2 half blob: tiles (ki_local=kj, mi), nm=6
                            nc.tensor.matmul(pf2s[mi][:],
                                             ff2_w[:, (kj*KT+mi)*128:(kj*KT+mi+1)*128],
                                             h_bf[:, kj*S:(kj+1)*S],
                                             start=(half == 0 and kj == 0),
                                             stop=(half == 1 and kj == 11))
                x_n = T(ap_, [128, KT*S], F32, "x0", f"x_l{l+1}")
                xbf_n = T(ap_, [128, KT*S], BF16, "xbf0", f"xbf_l{l+1}")
                for mi in range(KT):
                    t1 = T(ap_, [128, S], F32, "zt2", f"xo_{l}_{mi}", bufs=2)
                    nc.vector.tensor_scalar(out=t1[:], in0=pf2s[mi][:],
                                            scalar1=bias[:, FFB2+mi:FFB2+mi+1],
                                            scalar2=None, op0=OP.add)
                    nc.vector.tensor_tensor(out=x_n[:, mi*S:(mi+1)*S],
                                            in0=sm[:, mi*S:(mi+1)*S], in1=t1[:], op=OP.add)
                    nc.scalar.copy(out=xbf_n[:, mi*S:(mi+1)*S], in_=x_n[:, mi*S:(mi+1)*S])
                if dbg and l == 0:
                    nc.sync.dma_start(out=dbgT[:, 9216:10752], in_=x_n[:])
                x, x_bf = x_n, xbf_n

            # ================= memory scan =================
            zr_s = ld(zrww, [128, KT*32], BF16, "zrwS", "zrw_s")
            zw_s = ld(zwww, [128, KT*32], BF16, "zwwS", "zww_s")
            zm_s = ld(zmtw, [128, KT*32], BF16, "zmtS", "zmt_s")
            zmp_s = ld(zmpw, [128, 6*128], BF16, "zmpS", "zmp_s")
            zog_s = T(wp, [128, 72*128], BF16, "ff1w", "zog_s")
            nc.sync.dma_start(out=zog_s[:], in_=zogw[:])

            def small_mm(blob, act, name, ptag):
                psx = pp.tile([32, S], F32, tag=ptag, name=name)
                for ki in range(KT):
                    nc.tensor.matmul(psx[:], blob[:, ki*32:(ki+1)*32],
                                     act[:, ki*S:(ki+1)*S],
                                     start=(ki == 0), stop=(ki == KT-1))
                return psx

            prw = small_mm(zr_s, x_bf, "prw", "pr0")
            rw = T(rp, [32, S], F32, "r_rw", "rw_s")
            nc.scalar.activation(rw[:], prw[:], AF.Sigmoid,
                                 bias=sbias_s[0:32, ZRWB:ZRWB+1])
            pww = small_mm(zw_s, x_bf, "pww", "pr1")
            ww = T(rp, [32, S], F32, "r_ww", "ww_s")
            nc.scalar.activation(ww[:], pww[:], AF.Sigmoid,
                                 bias=sbias_s[0:32, ZWWB:ZWWB+1])
            pnm = small_mm(zm_s, x_bf, "pnm", "pr0")
            nmt = T(rp, [32, S], F32, "r_nmt", "nmt_s")
            nc.vector.tensor_scalar(out=nmt[:], in0=pnm[:],
                                    scalar1=sbias_s[0:32, ZMTB:ZMTB+1], scalar2=None,
                                    op0=OP.add)
            af = T(rp, [32, S], F32, "r_af", "af_s")
            nc.vector.tensor_scalar(out=af[:], in0=ww[:], scalar1=-1.0, scalar2=1.0,
                                    op0=OP.mult, op1=OP.add)
            bf_ = T(rp, [32, S], F32, "r_bf", "bf_s")
            nc.vector.tensor_tensor(out=bf_[:], in0=ww[:], in1=nmt[:], op=OP.mult)
            Msc = T(rp, [32, S], F32, "r_M", "M_s")
            nc.vector.tensor_tensor_scan(Msc[:], af[:], bf_[:], 0.0, OP.mult, OP.add)
            Mp = T(rp, [32, S], F32, "r_nmt", "Mp_s")
            nc.gpsimd.memset(Mp[:, 0:1], 0.0)
            nc.vector.tensor_copy(out=Mp[:, 1:S], in_=Msc[:, 0:S-1])
            rwm = T(rp, [32, S], F32, "r_af", "rwm_s")
            nc.vector.tensor_tensor(out=rwm[:], in0=rw[:], in1=Mp[:], op=OP.mult)
            rwm_b = T(rp, [32, S], BF16, "r_rwmb", "rwmb_s")
            nc.scalar.copy(out=rwm_b[:], in_=rwm[:])
            mv_bf = T(ap_, [128, KT*S], BF16, "lnb", "mvbf_s")
            for mi in range(KT):
                pmv = pp.tile([128, S], F32, tag=f"pm{mi}", name=f"mv_{mi}")
                nc.tensor.matmul(pmv[:], zmp_s[0:32, mi*128:(mi+1)*128], rwm_b[:],
                                 start=True, stop=True)
                t1 = T(ap_, [128, S], F32, "zt1", f"mvt_{mi}", bufs=2)
                nc.vector.tensor_scalar(out=t1[:], in0=pmv[:],
                                        scalar1=sbias_s[:, ZMPB+mi:ZMPB+mi+1],
                                        scalar2=None, op0=OP.add)
                nc.scalar.copy(out=mv_bf[:, mi*S:(mi+1)*S], in_=t1[:])
            fused = T(ap_, [128, KT*S], F32, "pre", "fused_s")
            for mi in range(KT):
                pfu = pp.tile([128, S], F32, tag=f"pm{mi}", name=f"fu_{mi}")
                for ki in range(12):
                    rhs = x_bf[:, ki*S:(ki+1)*S] if ki < KT else \
                        mv_bf[:, (ki-KT)*S:(ki-KT+1)*S]
                    nc.tensor.matmul(pfu[:], zog_s[:, (ki*KT+mi)*128:(ki*KT+mi+1)*128],
                                     rhs, start=(ki == 0), stop=(ki == 11))
                nc.scalar.activation(fused[:, mi*S:(mi+1)*S], pfu[:], AF.Tanh,
                                     bias=sbias_s[:, ZOGB+mi:ZOGB+mi+1])
            if dbg:
                nc.sync.dma_start(out=dbgT[:, 10752:12288], in_=fused[:])
            y_bf = T(ap_, [128, KT*S], BF16, "jzbf", "ybf_s")
            layernorm("hln", fused, sbias_s[:, HLNG:HLNG+6], sbias_s[:, HLNB:HLNB+6],
                      y_bf)

            if dbg:
                nc.sync.dma_start(out=dbgB[:, 9216:10752], in_=y_bf[:])
            # ================= head =================
            for mi in range(head_mt):
                hwt = T(wp, [128, KT*128], BF16, f"hw{mi % 2}", f"hw_{mi}")
                for ki in range(KT):
                    nc.sync.dma_start(
                        out=hwt[:, ki*128:(ki+1)*128],
                        in_=headw[:, (ki*HEADMT+mi)*128:(ki*HEADMT+mi+1)*128])
                ph = pp.tile([128, S], F32, tag=f"pm{mi % 6}", name=f"hd_{mi}")
                for ki in range(KT):
                    nc.tensor.matmul(ph[:], hwt[:, ki*128:(ki+1)*128],
                                     y_bf[:, ki*S:(ki+1)*S],
                                     start=(ki == 0), stop=(ki == KT-1))
                ob = T(ap_, [128, S], F32, f"ob{mi % 3}", f"ob_{mi}")
                nc.scalar.copy(out=ob[:], in_=ph[:])
                nc.sync.dma_start(out=out[:, mi*S:(mi+1)*S], in_=ob[:])

    nc.compile()
    return nc


_CACHE = {}


def _get_nc(n_layers, head_mt, meta, dbg=False):
    key = (n_layers, head_mt, dbg)
    if key not in _CACHE:
        _CACHE[key] = build(n_layers, head_mt, meta, dbg=dbg)
    return _CACHE[key]


def kernel(input_ids, params, n_layers=LFULL, head_mt=HEADMT, want_trace=False, dbg=False):
    from concourse.bass_utils import run_bass_kernel_spmd
    g, meta = prep_host(params, n_layers)
    nc = _get_nc(n_layers, head_mt, meta, dbg=dbg)
    ids = np.asarray(input_ids)
    in_maps = []
    for c in range(N_CORES):
        m = dict(g)
        m['ids'] = np.ascontiguousarray(
            ids[c].astype(np.int32).reshape(2, 128).T)
        in_maps.append(m)
    trace = False
    if want_trace:
        try:
            import ntff_shim
            ntff_shim.install()
            trace = True
        except Exception:
            pass
    if trace:
        # axon NTFF profiling needs an initialized PJRT client: warm run first
        run_bass_kernel_spmd(nc, in_maps, list(range(N_CORES)), trace=False)
        try:
            res = run_bass_kernel_spmd(nc, in_maps, list(range(N_CORES)), trace=True)
        except Exception as e:
            print("trace failed:", e)
            res = run_bass_kernel_spmd(nc, in_maps, list(range(N_CORES)), trace=False)
    else:
        res = run_bass_kernel_spmd(nc, in_maps, list(range(N_CORES)), trace=False)
    outs = []
    for c in range(N_CORES):
        blob = res.results[c]["out"]
        outs.append(blob.reshape(128, head_mt, S).transpose(2, 1, 0)
                    .reshape(S, head_mt*128))
    logits = np.stack(outs, 0).astype(np.float32)
    kernel.last_exec_ns = res.exec_time_ns
    if dbg:
        kernel.dbg = res.results[0].get("dbg")
        kernel.dbgb = res.results[0].get("dbgb")
    return logits


# revision 16
# speedup vs baseline: 11.1235x; 11.1235x over previous
"""BaGuaLLM Trainium2 kernel: 8-core batch-parallel, feature-major layout.

Activations live feature-major in SBUF: [128, KT*256] where global feature
f = ki*128 + p sits at [p, ki*256 + t]. Matmuls contract features on the
partition dim; weights are host-packed tile-major bf16. Cross-core traffic:
one 64B AllReduce per layer (head-elimination stats) plus one for the
scene/causal flag.
"""
import sys
sys.path.insert(0, '/opt/trn_rl_repo')
import numpy as np
import ml_dtypes

BF = ml_dtypes.bfloat16
N_CORES = 8
DIM, KT, S, NH, HD, FF, VOCAB, LFULL = 768, 6, 256, 8, 96, 3072, 32000, 12
HEADMT = VOCAB // 128  # 250
THR = 0.3
SCENE_W = np.array([0.95,0.95,0.9,0.85,0.8,0.95,0.9,0.05,0.05,0.1,0.1,0.05,
                    0.05,0.05,0.1,0.05,0.4,0.7,0.35,0.45,0.6,0.5,0.65],
                   dtype=np.float32)

# per-layer bias blob columns
OUT_B, FFB1, FFB2, LN1G, LN1B, FFLNG, FFLNB, SH1B, SH2B, POLB = \
    0, 6, 30, 36, 42, 48, 54, 60, 63, 64
IMPW1, IMPB1, IMPW2, IMPB2 = 96, 224, 352, 480
NBIAS = 481
# static bias blob columns
ZRWB, ZWWB, ZMTB, ZMPB, ZOGB, HLNG, HLNB, TSB1, TSB2, CGB, DGB = \
    0, 1, 2, 3, 9, 15, 21, 27, 30, 31, 37
EPSC, IGB, NEG1 = 43, 44, 45
NSBIAS = 46


def tilemaj(w):
    K, M = w.shape
    nk = K // 128
    if M % 128:
        w = np.concatenate([w, np.zeros((K, 128 - M % 128), w.dtype)], 1)
    nm = w.shape[1] // 128
    return np.ascontiguousarray(
        w.reshape(nk, 128, nm, 128).transpose(1, 0, 2, 3).reshape(128, nk*nm*128)), nk, nm


def tilemaj_narrow(w):
    K, M = w.shape
    nk = K // 128
    return np.ascontiguousarray(
        w.reshape(nk, 128, M).transpose(1, 0, 2).reshape(128, nk*M))


def ktile(w):
    K, N = w.shape
    nk = K // 128
    return np.ascontiguousarray(
        w.reshape(nk, 128, N).transpose(1, 0, 2).reshape(128, nk*N))


def colpack(v):
    return np.ascontiguousarray(v.reshape(-1, 128).T)


def pack_banded(w, nk, nm):
    slots, smap = [], []
    for mi in range(nm):
        ks = [ki for ki in range(nk)
              if np.abs(w[ki*128:(ki+1)*128, mi*128:(mi+1)*128]).max() > 0]
        ks = ks if ks else [0]
        smap.append([(ki, len(slots) + j) for j, ki in enumerate(ks)])
        slots.extend((ki, mi) for ki in ks)
    blob = np.zeros((128, len(slots)*128), w.dtype)
    for si, (ki, mi) in enumerate(slots):
        blob[:, si*128:(si+1)*128] = w[ki*128:(ki+1)*128, mi*128:(mi+1)*128]
    return blob, smap


def seg_rows(n):
    segs = []
    r0, r1 = n*96, (n+1)*96
    ki = r0 // 128
    while r0 < r1:
        hi = min(r1, (ki+1)*128)
        segs.append((ki, r0 - ki*128, hi - ki*128, r0 - n*96))
        r0 = hi
        ki += 1
    return segs


def prep_host(params, n_layers):
    p = {k: np.asarray(v) for k, v in params.items()}
    g, meta = {}, {}
    g['emb'] = np.ascontiguousarray(p['embedding'].astype(np.float32))

    trig_l, out_l, ff1_l, ff2_l, sh1_l, sh2_l, pol_l, bias_l = ([] for _ in range(8))
    for l in range(n_layers):
        tw = p['trig_w'][l] * np.cos(p['res_freq'][l] * np.pi)[:, None, :]
        TW = tw.transpose(1, 0, 2).reshape(DIM, DIM)
        trig_l.append(tilemaj(TW.astype(BF))[0])
        out_l.append(tilemaj(p['out_w'][l].astype(BF))[0])
        W1 = p['ff_w1'][l].astype(BF)
        ff1_l.append(np.concatenate(
            [tilemaj(W1[:, h*1536:(h+1)*1536])[0] for h in range(2)], 1))
        W2 = p['ff_w2'][l].astype(BF)
        ff2_l.append(np.concatenate(
            [tilemaj(W2[h*1536:(h+1)*1536, :])[0] for h in range(2)], 1))
        S1 = np.zeros((DIM, NH*48), np.float32)
        for n in range(NH):
            S1[n*96:(n+1)*96, n*48:(n+1)*48] = p['sh_w1'][l, n]
        blob1, smap1 = pack_banded(S1.astype(BF), KT, 3)
        sh1_l.append(blob1)
        if l == 0:
            meta['sh1_map'] = smap1
            meta['sh1_slots'] = blob1.shape[1] // 128
        S2 = np.zeros((NH*48, NH), np.float32)
        for n in range(NH):
            S2[n*48:(n+1)*48, n] = p['sh_w2'][l, n, :, 0]
        sh2_l.append(tilemaj_narrow(S2.astype(BF)))
        PS = np.zeros((DIM, 32), np.float32)
        for n in range(NH):
            PS[n*96:(n+1)*96, :] = p['pol_w'][l, n] / S
        pol_l.append(ktile(PS.astype(BF)))
        b = np.zeros((128, NBIAS), np.float32)
        b[:, OUT_B:OUT_B+6] = colpack(p['out_b'][l])
        b[:, FFB1:FFB1+24] = colpack(p['ff_b1'][l])
        b[:, FFB2:FFB2+6] = colpack(p['ff_b2'][l])
        b[:, LN1G:LN1G+6] = colpack(p['ln1_g'][l])
        b[:, LN1B:LN1B+6] = colpack(p['ln1_b'][l])
        b[:, FFLNG:FFLNG+6] = colpack(p['ff_ln_g'][l])
        b[:, FFLNB:FFLNB+6] = colpack(p['ff_ln_b'][l])
        b[:, SH1B:SH1B+3] = np.ascontiguousarray(
            p['sh_b1'][l].reshape(-1).reshape(3, 128).T)
        b[:8, SH2B] = p['sh_b2'][l, :, 0]
        b[:8, POLB:POLB+32] = p['pol_b'][l]
        b[:8, IMPW1:IMPW1+128] = np.tile(p['imp_w1'][l, 0], 8)[None, :].repeat(8, 0)
        b[:8, IMPB1:IMPB1+128] = np.tile(p['imp_b1'][l], 8)[None, :].repeat(8, 0)
        b[:8, IMPW2:IMPW2+128] = np.tile(p['imp_w2'][l, :, 0], 8)[None, :].repeat(8, 0)
        b[:8, IMPB2] = p['imp_b2'][l, 0]
        bias_l.append(b)
    cat = lambda xs: np.ascontiguousarray(np.concatenate(xs, 1))
    g['trig'], g['outw'] = cat(trig_l), cat(out_l)
    g['ff1w'], g['ff2w'] = cat(ff1_l), cat(ff2_l)
    g['sh1w'], g['sh2w'], g['polw'], g['biasb'] = \
        cat(sh1_l), cat(sh2_l), cat(pol_l), cat(bias_l)

    Z = np.zeros((DIM, DIM), np.float32)
    for n in range(NH):
        Z[n*96:(n+1)*96, n*96:(n+1)*96] = p['zone_w'][n, :96, :]
    zb, zmap = pack_banded(Z.astype(BF), KT, KT)
    g['zonew'] = zb
    meta['zone_map'] = zmap
    meta['zone_slots'] = zb.shape[1] // 128
    pos = np.arange(S, dtype=np.float32) / (S - 1)
    zs = S / NH
    zr = (np.arange(S, dtype=np.float32) % zs) / zs
    tc_ = np.arange(NH, dtype=np.float32) / 7.0
    pc = np.stack([pos[None, :]*0.5 + tc_[:, None]*0.5,
                   np.broadcast_to(zr, (NH, S))], -1)
    Z2 = np.zeros((16, DIM), np.float32)
    for n in range(NH):
        Z2[n*2:(n+1)*2, n*96:(n+1)*96] = p['zone_w'][n, 96:98, :]
    g['zw2'] = np.ascontiguousarray(Z2.reshape(16, KT, 128).reshape(16, KT*128)).astype(BF)
    g['pc16'] = np.ascontiguousarray(
        pc.transpose(0, 2, 1).reshape(16, S)).astype(BF)
    G = np.zeros((DIM, NH), np.float32)
    for n in range(NH):
        G[n*96:(n+1)*96, n] = p['ig_w'][:, 0]
    g['igw'] = tilemaj_narrow(G.astype(BF))
    g['cgw'] = tilemaj(p['buf_cg_w'].astype(BF))[0]
    g['dgw'] = tilemaj(p['buf_dg_w'].astype(BF))[0]
    g['posT'] = ktile(np.ascontiguousarray(p['pos_embedding'][:S].T).astype(np.float32))
    g['tsa1'] = tilemaj(p['tsa_w1'].astype(BF))[0]
    g['tsa2'] = tilemaj_narrow(p['tsa_w2'].astype(BF))
    g['zrww'] = tilemaj_narrow(p['z_rw_w'].astype(BF))
    g['zwww'] = tilemaj_narrow(p['z_ww_w'].astype(BF))
    g['zmtw'] = tilemaj_narrow(p['z_mt_w'].astype(BF))
    zmp = np.zeros((128, 6*128), BF)
    zmp[:32] = np.ascontiguousarray(p['z_mp_w']).astype(BF).reshape(32, 768)
    g['zmpw'] = zmp
    g['zogw'] = tilemaj(p['z_og_w'].astype(BF))[0]
    g['headw'] = tilemaj(p['head_w'].astype(BF))[0]
    fr = np.arange(NH*96) // 96
    e8 = (fr[None, :] == np.arange(NH)[:, None]).astype(np.float32)
    g['e8bf'] = e8.astype(BF)
    e8c = np.zeros((128, KT*8), np.float32)
    ist = np.zeros((128, KT*96), np.float32)
    for ki in range(KT):
        rows = np.arange(128) + ki*128
        e8c[:, ki*8:(ki+1)*8] = (rows[:, None]//96 == np.arange(8)[None, :])
        ist[:, ki*96:(ki+1)*96] = ((rows % 96)[:, None] == np.arange(96)[None, :])
    g['e8c'] = e8c
    g['e8cb'] = e8c.astype(BF)
    g['istack'] = ist.astype(BF)
    g['eyec'] = (1.0 - np.eye(8)).astype(np.float32)
    g['pre8'] = (0.1 * (1.0 - np.eye(8))).astype(np.float32)
    scw = np.zeros((1, 32), np.float32)
    scw[0, :23] = SCENE_W
    g['scw'] = scw
    g['causm1'] = np.ascontiguousarray(
        ((np.arange(S, dtype=np.float32)+1.0)/S - 1.0)[None, :])
    g['onesb'] = np.ones((128, 1), BF)
    sb = np.zeros((128, NSBIAS), np.float32)
    sb[:32, ZRWB] = p['z_rw_b']
    sb[:32, ZWWB] = p['z_ww_b']
    sb[:32, ZMTB] = p['z_mt_b']
    sb[:, ZMPB:ZMPB+6] = colpack(p['z_mp_b'])
    sb[:, ZOGB:ZOGB+6] = colpack(p['z_og_b'])
    sb[:, HLNG:HLNG+6] = colpack(p['head_ln_g'])
    sb[:, HLNB:HLNB+6] = colpack(p['head_ln_b'])
    sb[:, TSB1:TSB1+3] = np.ascontiguousarray(p['tsa_b1'].reshape(3, 128).T)
    sb[:23, TSB2] = p['tsa_b2']
    sb[:, CGB:CGB+6] = colpack(p['buf_cg_b'])
    sb[:, DGB:DGB+6] = colpack(p['buf_dg_b'])
    sb[:, EPSC] = 1e-5
    sb[:, IGB] = float(p['ig_b'][0])
    sb[:, NEG1] = -1.0
    g['sbias'] = sb
    meta['ig_b'] = float(p['ig_b'][0])
    return g, meta


def build(n_layers, head_mt, meta, dbg=False):
    from concourse import bass, mybir, tile, bacc
    from concourse.masks import make_identity
    dt = mybir.dt
    F32, BF16, I32 = dt.float32, dt.bfloat16, dt.int32
    AF = mybir.ActivationFunctionType
    OP = mybir.AluOpType
    ig_b = meta['ig_b']

    nc = bacc.Bacc("TRN2", target_bir_lowering=False, debug=False,
                   num_devices=N_CORES)
    D = lambda name, shape, dty: nc.dram_tensor(name, shape, dty,
                                                kind="ExternalInput").ap()
    emb = D("emb", [VOCAB, DIM], F32)
    ids = D("ids", [128, 2], I32)
    trig = D("trig", [128, n_layers*36*128], BF16)
    outw = D("outw", [128, n_layers*36*128], BF16)
    ff1w = D("ff1w", [128, n_layers*144*128], BF16)
    ff2w = D("ff2w", [128, n_layers*144*128], BF16)
    NSH1 = meta["sh1_slots"]
    sh1w = D("sh1w", [128, n_layers*NSH1*128], BF16)
    sh2w = D("sh2w", [128, n_layers*24], BF16)
    polw = D("polw", [128, n_layers*192], BF16)
    biasb = D("biasb", [128, n_layers*NBIAS], F32)
    NZONE = meta["zone_slots"]
    zonew = D("zonew", [128, NZONE*128], BF16)
    zw2 = D("zw2", [16, KT*128], BF16)
    pc16 = D("pc16", [16, S], BF16)
    igw = D("igw", [128, KT*8], BF16)
    cgw = D("cgw", [128, 36*128], BF16)
    dgw = D("dgw", [128, 36*128], BF16)
    posT = D("posT", [128, KT*S], F32)
    tsa1 = D("tsa1", [128, 18*128], BF16)
    tsa2 = D("tsa2", [128, 3*23], BF16)
    zrww = D("zrww", [128, KT*32], BF16)
    zwww = D("zwww", [128, KT*32], BF16)
    zmtw = D("zmtw", [128, KT*32], BF16)
    zmpw = D("zmpw", [128, 6*128], BF16)
    zogw = D("zogw", [128, 72*128], BF16)
    headw = D("headw", [128, KT*HEADMT*128], BF16)
    e8bf = D("e8bf", [8, DIM], BF16)
    e8c = D("e8c", [128, KT*8], F32)
    e8cb = D("e8cb", [128, KT*8], BF16)
    istack = D("istack", [128, KT*96], BF16)
    eyec = D("eyec", [8, 8], F32)
    pre8 = D("pre8", [8, 8], F32)
    scw = D("scw", [1, 32], F32)
    causm1 = D("causm1", [1, S], F32)
    onesb = D("onesb", [128, 1], BF16)
    sbias = D("sbias", [128, NSBIAS], F32)
    out = nc.dram_tensor("out", [128, head_mt*S], F32, kind="ExternalOutput").ap()
    if dbg:
        dbgT = nc.dram_tensor("dbg", [128, 12416], F32, kind="ExternalOutput").ap()
        dbgB = nc.dram_tensor("dbgb", [128, 10752], BF16, kind="ExternalOutput").ap()

    nAR = n_layers + 1
    arin = [nc.dram_tensor(f"ari{i}", [64, 1], F32).ap() for i in range(nAR)]
    arout = [nc.dram_tensor(f"aro{i}", [64, 1], F32, addr_space="Shared").ap()
             for i in range(nAR)]

    with tile.TileContext(nc) as tc:
        import contextlib
        with contextlib.ExitStack() as ctx:
            sp = ctx.enter_context(tc.tile_pool(name="static", bufs=1))
            wp = ctx.enter_context(tc.tile_pool(name="wstream", bufs=1))
            ap_ = ctx.enter_context(tc.tile_pool(name="acts", bufs=1))
            rp = ctx.enter_context(tc.tile_pool(name="rows", bufs=1))
            pp = ctx.enter_context(tc.tile_pool(name="ps", bufs=1, space="PSUM"))

            def T(pool, shape, dty, tag, name, bufs=1):
                return pool.tile(shape, dty, tag=tag, name=name, bufs=bufs)

            def ld(dram, shape, dty, tag, name):
                t = T(sp, shape, dty, tag, name)
                nc.sync.dma_start(out=t[:], in_=dram[:])
                return t

            zone_s = ld(zonew, [128, NZONE*128], BF16, "zone", "zone_s")
            zw2_s = ld(zw2, [16, KT*128], BF16, "zw2S", "zw2_s")
            pc16_s = ld(pc16, [16, S], BF16, "pc16S", "pc16_s")
            igw_s = ld(igw, [128, KT*8], BF16, "igwS", "igw_s")
            cgw_s = ld(cgw, [128, 36*128], BF16, "cgwS", "cgw_s")
            dgw_s = ld(dgw, [128, 36*128], BF16, "dgwS", "dgw_s")
            e8bf_s = ld(e8bf, [8, DIM], BF16, "e8bfS", "e8bf_s")
            e8c_s = ld(e8c, [128, KT*8], F32, "e8cS", "e8c_s")
            e8cb_s = ld(e8cb, [128, KT*8], BF16, "e8cbS", "e8cb_s")
            ist_s = ld(istack, [128, KT*96], BF16, "istS", "ist_s")
            eyec_s = ld(eyec, [8, 8], F32, "eyecS", "eyec_s")
            pre8_s = ld(pre8, [8, 8], F32, "pre8S", "pre8_s")
            scw_s = ld(scw, [1, 32], F32, "scwS", "scw_s")
            causm1_s = ld(causm1, [1, S], F32, "causS", "causm1_s")
            ones_s = ld(onesb, [128, 1], BF16, "onesS", "ones_s")
            sbias_s = ld(sbias, [128, NSBIAS], F32, "sbiasS", "sbias_s")
            ident = T(sp, [128, 128], F32, "identS", "ident_s")
            make_identity(nc, ident[:])
            # scratch 32x32 tiles
            pv8 = T(sp, [32, 32], F32, "pv8S", "pv8_s")
            pvT = T(sp, [32, 32], F32, "pvTS", "pvT_s")
            i32t = T(sp, [32, 32], F32, "i32S", "i32_s")
            it32 = T(sp, [32, 32], F32, "it32S", "it32_s")
            c32 = T(sp, [32, 32], F32, "c32S", "c32_s")
            ct32 = T(sp, [32, 32], F32, "ct32S", "ct32_s")
            z23 = T(sp, [32, 32], F32, "z23S", "z23_s")
            zt = T(sp, [32, 32], F32, "ztS", "zt_s")
            fbc = T(sp, [128, S], F32, "fbcS", "fbc_s")

            # ---- embedding gather -> feature-major x
            ids_sb = T(sp, [128, 2], I32, "idsS", "ids_sb")
            nc.sync.dma_start(out=ids_sb[:], in_=ids[:])
            x = T(ap_, [128, KT*S], F32, "x0", "x_l0")
            for hi in range(2):
                gx = T(ap_, [128, DIM], F32, "gx", f"gx{hi}")
                nc.gpsimd.indirect_dma_start(
                    out=gx[:], out_offset=None, in_=emb[:],
                    in_offset=bass.IndirectOffsetOnAxis(ap=ids_sb[:, hi:hi+1], axis=0))
                for fi in range(KT):
                    pt = pp.tile([128, S], F32, tag=f"pm{fi}", name=f"tp{hi}_{fi}")
                    nc.tensor.transpose(out=pt[:, :128], in_=gx[:, fi*128:(fi+1)*128],
                                        identity=ident[:])
                    nc.scalar.copy(out=x[:, fi*S+hi*128:fi*S+hi*128+128],
                                   in_=pt[:, :128])
            pos_s = T(wp, [128, 36*128], F32, "outw", "pos_s")
            nc.sync.dma_start(out=pos_s[:, :KT*S], in_=posT[:])
            for ki in range(KT):
                nc.vector.tensor_tensor(out=x[:, ki*S:(ki+1)*S], in0=x[:, ki*S:(ki+1)*S],
                                        in1=pos_s[:, ki*S:(ki+1)*S], op=OP.add)
            x_bf = T(ap_, [128, KT*S], BF16, "xbf0", "xbf_l0")
            for ki in range(KT):
                nc.scalar.copy(out=x_bf[:, ki*S:(ki+1)*S], in_=x[:, ki*S:(ki+1)*S])
            if dbg:
                nc.sync.dma_start(out=dbgT[:, 0:1536], in_=x[:])

            # ---- scene -> use_causal -> factor broadcast
            tsa1_s = T(wp, [128, 36*128], BF16, "trigw", "tsa1_s")
            nc.sync.dma_start(out=tsa1_s[:, :18*128], in_=tsa1[:])
            tsa2_s = T(sp, [128, 3*23], BF16, "tsa2S", "tsa2_s")
            nc.sync.dma_start(out=tsa2_s[:], in_=tsa2[:])
            tg = T(sp, [128, 3], BF16, "tgS", "tg_s")
            for mi in range(3):
                pt = pp.tile([128, 1], F32, tag="pr0", name=f"tsa_{mi}")
                for ki in range(KT):
                    nc.tensor.matmul(pt[:], tsa1_s[:, (ki*3+mi)*128:(ki*3+mi+1)*128],
                                     x_bf[:, ki*S:ki*S+1],
                                     start=(ki == 0), stop=(ki == KT-1))
                nc.scalar.activation(tg[:, mi:mi+1], pt[:], AF.Gelu,
                                     bias=sbias_s[:, TSB1+mi:TSB1+mi+1])
            nc.gpsimd.memset(z23[:], 0.0)
            pz = pp.tile([23, 1], F32, tag="pr0", name="pz23")
            for ki in range(3):
                nc.tensor.matmul(pz[:], tsa2_s[:, ki*23:(ki+1)*23], tg[:, ki:ki+1],
                                 start=(ki == 0), stop=(ki == 2))
            nc.vector.tensor_scalar(out=z23[0:23, 0:1], in0=pz[:],
                                    scalar1=sbias_s[0:23, TSB2:TSB2+1], scalar2=None,
                                    op0=OP.add)
            nc.vector.transpose(zt[:], z23[:])
            nc.gpsimd.memset(zt[0:1, 23:32], -1e30)
            mx = T(rp, [1, 1], F32, "r_mx", "sc_mx")
            nc.vector.tensor_reduce(mx[:], zt[0:1, :], axis=mybir.AxisListType.X, op=OP.max)
            exr = T(rp, [1, 32], F32, "r_ex", "sc_ex")
            nc.vector.tensor_scalar(out=exr[:], in0=zt[0:1, :], scalar1=mx[:, :],
                                    scalar2=None, op0=OP.subtract)
            nc.scalar.activation(exr[:], exr[:], AF.Exp)
            smr = T(rp, [1, 1], F32, "r_sm", "sc_sm")
            nc.vector.tensor_reduce(smr[:], exr[:], axis=mybir.AxisListType.X, op=OP.add)
            rsr = T(rp, [1, 1], F32, "r_rs", "sc_rs")
            nc.vector.reciprocal(rsr[:], smr[:])
            nc.vector.tensor_scalar(out=exr[:], in0=exr[:], scalar1=rsr[:, :],
                                    scalar2=None, op0=OP.mult)
            nc.vector.tensor_tensor(out=exr[:], in0=exr[:], in1=scw_s[:], op=OP.mult)
            ucp = T(rp, [1, 1], F32, "r_ucp", "sc_ucp")
            nc.vector.tensor_reduce(ucp[:], exr[:], axis=mybir.AxisListType.X, op=OP.add)
            nc.sync.dma_start(out=arin[n_layers][0:1, :], in_=ucp[:])
            nc.gpsimd.collective_compute(
                "AllReduce", OP.add, ins=[arin[n_layers][:]], outs=[arout[n_layers][:]],
                replica_groups=[list(range(N_CORES))])
            ucs = T(rp, [1, 1], F32, "r_ucs", "sc_ucs")
            nc.sync.dma_start(out=ucs[:], in_=arout[n_layers][0:1, :])
            ucf = T(rp, [1, 1], F32, "r_ucf", "sc_ucf")
            nc.vector.tensor_scalar(out=ucf[:], in0=ucs[:], scalar1=4.0, scalar2=None,
                                    op0=OP.is_gt)
            frow = T(rp, [1, S], F32, "r_frow", "sc_frow")
            nc.vector.tensor_scalar(out=frow[:], in0=causm1_s[:], scalar1=ucf[:, :],
                                    scalar2=1.0, op0=OP.mult, op1=OP.add)
            nc.gpsimd.partition_broadcast(fbc[:], frow[:])

            heat_i = [0]
            def heat(n_heat):
                for _ in range(n_heat):
                    hp = pp.tile([128, S], F32, tag="pm4", name=f"heat_{heat_i[0]}")
                    heat_i[0] += 1
                    nc.tensor.matmul(hp[:], ident[:], fbc[:], start=True, stop=True)

            def mm_block(psum, blob, nm, mi, rhs, kis):
                for j, ki in enumerate(kis):
                    nc.tensor.matmul(psum, blob[:, (ki*nm+mi)*128:(ki*nm+mi+1)*128],
                                     rhs[:, ki*S:(ki+1)*S],
                                     start=(j == 0), stop=(j == len(kis)-1))

            def layernorm(lname, src, gcol, bcol, out_t):
                srcb = T(ap_, [128, KT*S], BF16, "lnb", f"{lname}_srcb")
                sqb = T(ap_, [128, KT*S], BF16, "lnsq", f"{lname}_sqb")
                for ki in range(KT):
                    nc.scalar.copy(out=srcb[:, ki*S:(ki+1)*S], in_=src[:, ki*S:(ki+1)*S])
                    nc.scalar.activation(sqb[:, ki*S:(ki+1)*S], srcb[:, ki*S:(ki+1)*S],
                                         AF.Square)
                pA = pp.tile([1, S], F32, tag="pr0", name=f"{lname}_pA")
                pB = pp.tile([1, S], F32, tag="pr1", name=f"{lname}_pB")
                for ki in range(KT):
                    nc.tensor.matmul(pA[:], ones_s[:, 0:1], srcb[:, ki*S:(ki+1)*S],
                                     start=(ki == 0), stop=(ki == KT-1))
                for ki in range(KT):
                    nc.tensor.matmul(pB[:], ones_s[:, 0:1], sqb[:, ki*S:(ki+1)*S],
                                     start=(ki == 0), stop=(ki == KT-1))
                m = T(rp, [1, S], F32, "r_lm", f"{lname}_m")
                e2 = T(rp, [1, S], F32, "r_le", f"{lname}_e2")
                nc.scalar.activation(m[:], pA[:], AF.Copy, scale=1.0/DIM)
                nc.scalar.activation(e2[:], pB[:], AF.Copy, scale=1.0/DIM)
                var = T(rp, [1, S], F32, "r_lv", f"{lname}_var")
                nc.vector.tensor_tensor(out=var[:], in0=m[:], in1=m[:], op=OP.mult)
                nc.vector.tensor_tensor(out=var[:], in0=e2[:], in1=var[:], op=OP.subtract)
                sd = T(rp, [1, S], F32, "r_le", f"{lname}_sd")
                nc.scalar.activation(sd[:], var[:], AF.Sqrt,
                                     bias=sbias_s[0:1, EPSC:EPSC+1])
                arow = T(rp, [1, S], F32, "r_la", f"{lname}_ar")
                nc.vector.reciprocal(arow[:], sd[:])
                brow = T(rp, [1, S], F32, "r_lv", f"{lname}_br")
                nc.vector.tensor_tensor(out=brow[:], in0=m[:], in1=arow[:], op=OP.mult)
                nc.vector.tensor_scalar(out=brow[:], in0=brow[:], scalar1=-1.0,
                                        scalar2=None, op0=OP.mult)
                aB = T(ap_, [128, S], F32, "aB", f"{lname}_aB")
                bB = T(ap_, [128, S], F32, "bB", f"{lname}_bB")
                nc.gpsimd.partition_broadcast(aB[:], arow[:])
                nc.gpsimd.partition_broadcast(bB[:], brow[:])
                for ki in range(KT):
                    t1 = T(ap_, [128, S], F32, "lnt", f"{lname}_t{ki}", bufs=2)
                    nc.vector.tensor_tensor(out=t1[:], in0=src[:, ki*S:(ki+1)*S],
                                            in1=aB[:], op=OP.mult)
                    nc.vector.tensor_tensor(out=t1[:], in0=t1[:], in1=bB[:], op=OP.add)
                    nc.vector.tensor_scalar(out=out_t[:, ki*S:(ki+1)*S], in0=t1[:],
                                            scalar1=gcol[:, ki:ki+1],
                                            scalar2=bcol[:, ki:ki+1],
                                            op0=OP.mult, op1=OP.add)

            # ================= layers =================
            for l in range(n_layers):
                trig_w = T(wp, [128, 36*128], BF16, "trigw", f"trig_{l}")
                nc.sync.dma_start(out=trig_w[:], in_=trig[:, l*36*128:(l+1)*36*128])
                out_w = T(wp, [128, 36*128], BF16, "outw", f"outw_{l}")
                nc.sync.dma_start(out=out_w[:], in_=outw[:, l*36*128:(l+1)*36*128])
                sh1_w = T(wp, [128, NSH1*128], BF16, "sh1w", f"sh1w_{l}")
                nc.sync.dma_start(out=sh1_w[:],
                                  in_=sh1w[:, l*NSH1*128:(l+1)*NSH1*128])
                sh2_w = T(wp, [128, 24], BF16, "sh2w", f"sh2w_{l}")
                nc.sync.dma_start(out=sh2_w[:], in_=sh2w[:, l*24:(l+1)*24])
                pol_w = T(wp, [128, 192], BF16, "polw", f"polw_{l}")
                nc.sync.dma_start(out=pol_w[:], in_=polw[:, l*192:(l+1)*192])
                bias = T(wp, [128, NBIAS], F32, "biasb", f"bias_{l}")
                nc.sync.dma_start(out=bias[:], in_=biasb[:, l*NBIAS:(l+1)*NBIAS])

                heads_bf = T(ap_, [128, KT*S], BF16, "heads", f"heads_{l}")
                hf_bf = T(ap_, [128, KT*S], BF16, "hf", f"hf_{l}")
                hm = T(ap_, [128, KT], F32, "hm", f"hm_{l}")
                for mi in range(KT):
                    ps = pp.tile([128, S], F32, tag=f"pm{mi}", name=f"trig_{l}_{mi}")
                    mm_block(ps[:], trig_w, KT, mi, x_bf, range(KT))
                    nc.scalar.copy(out=heads_bf[:, mi*S:(mi+1)*S], in_=ps[:])
                    nc.vector.tensor_tensor(out=hf_bf[:, mi*S:(mi+1)*S],
                                            in0=heads_bf[:, mi*S:(mi+1)*S], in1=fbc[:],
                                            op=OP.mult)
                    nc.vector.tensor_reduce(hm[:, mi:mi+1], heads_bf[:, mi*S:(mi+1)*S],
                                            axis=mybir.AxisListType.X, op=OP.add)
                if dbg and l == 0:
                    nc.sync.dma_start(out=dbgB[:, 0:1536], in_=heads_bf[:])
                    nc.sync.dma_start(out=dbgB[:, 1536:3072], in_=hf_bf[:])
                    nc.sync.dma_start(out=dbgT[:, 12345:12351], in_=hm[:])
                polL = T(ap_, [128, KT*8], BF16, "polL", f"polL_{l}")
                for ki in range(KT):
                    nc.vector.tensor_scalar(out=polL[:, ki*8:(ki+1)*8],
                                            in0=e8cb_s[:, ki*8:(ki+1)*8],
                                            scalar1=hm[:, ki:ki+1], scalar2=None,
                                            op0=OP.mult)
                heat(4)
                ppv = pp.tile([8, 32], F32, tag="pr0", name=f"pv_{l}")
                for ki in range(KT):
                    nc.tensor.matmul(ppv[:], polL[:, ki*8:(ki+1)*8],
                                     pol_w[:, ki*32:(ki+1)*32],
                                     start=(ki == 0), stop=(ki == KT-1))
                nc.gpsimd.memset(pv8[:], 0.0)
                nc.vector.tensor_tensor(out=pv8[0:8, :], in0=ppv[:],
                                        in1=bias[0:8, POLB:POLB+32], op=OP.add)
                nc.scalar.activation(pv8[0:8, :], pv8[0:8, :], AF.Tanh)
                if dbg and l == 0:
                    nc.sync.dma_start(out=dbgT[0:32, 12352:12384], in_=pv8[:])
                sq8 = T(rp, [8, 32], F32, "r_sq8", f"sq8_{l}")
                nc.vector.tensor_tensor(out=sq8[:], in0=pv8[0:8, :], in1=pv8[0:8, :],
                                        op=OP.mult)
                dnm = T(rp, [8, 1], F32, "r_dnm", f"dnm_{l}")
                nc.vector.tensor_reduce(dnm[:], sq8[:], axis=mybir.AxisListType.X, op=OP.add)
                nc.scalar.activation(dnm[:], dnm[:], AF.Sqrt)
                nc.vector.tensor_scalar(out=dnm[:], in0=dnm[:], scalar1=1e-12,
                                        scalar2=None, op0=OP.max)
                inv = T(rp, [8, 1], F32, "r_inv", f"inv_{l}")
                nc.vector.reciprocal(inv[:], dnm[:])
                nc.vector.transpose(pvT[:], pv8[:])
                pG = pp.tile([8, 8], F32, tag="pr1", name=f"G_{l}")
                nc.tensor.matmul(pG[:], pvT[:, 0:8], pvT[:, 0:8], start=True, stop=True)
                dpa = T(rp, [8, 8], F32, "r_dpa", f"dpa_{l}")
                nc.vector.tensor_scalar(out=dpa[:], in0=pG[:], scalar1=inv[:, :],
                                        scalar2=None, op0=OP.mult)
                nc.gpsimd.memset(i32t[:], 0.0)
                nc.vector.tensor_copy(out=i32t[0:8, 0:1], in_=inv[:])
                nc.vector.transpose(it32[:], i32t[:])
                invb = T(rp, [8, 8], F32, "r_invb", f"invb_{l}")
                nc.gpsimd.partition_broadcast(invb[:], it32[0:1, 0:8])
                dp = T(rp, [8, 8], F32, "r_dp", f"dp_{l}")
                nc.vector.tensor_tensor(out=dp[:], in0=dpa[:], in1=invb[:], op=OP.mult)
                if dbg and l == 0:
                    nc.sync.dma_start(out=dbgT[0:8, 12384:12392], in_=dp[:])
                h1 = T(rp, [8, 128], F32, "r_h1", f"h1_{l}")
                dpb = dp[:, :].rearrange("p (f o) -> p f o", o=1).to_broadcast([8, 8, 16])
                nc.vector.tensor_tensor(out=h1[:], in0=dpb,
                                        in1=bias[0:8, IMPW1:IMPW1+128], op=OP.mult)
                nc.vector.tensor_tensor(out=h1[:], in0=h1[:],
                                        in1=bias[0:8, IMPB1:IMPB1+128], op=OP.add)
                nc.scalar.activation(h1[:], h1[:], AF.Gelu)
                nc.vector.tensor_tensor(out=h1[:], in0=h1[:],
                                        in1=bias[0:8, IMPW2:IMPW2+128], op=OP.mult)
                impv = T(rp, [8, 8], F32, "r_imp", f"impv_{l}")
                nc.vector.tensor_reduce(impv[:],
                                        h1[:, :].rearrange("p (a b) -> p a b", b=16),
                                        axis=mybir.AxisListType.X, op=OP.add)
                nc.vector.tensor_scalar(out=impv[:], in0=impv[:],
                                        scalar1=bias[0:8, IMPB2:IMPB2+1], scalar2=None,
                                        op0=OP.add)
                # softplus(z) = ln(1+e^z) via Newton: w <- w - 1 + y*exp(-w)
                yv = T(rp, [8, 8], F32, "r_spy", f"spy_{l}")
                nc.scalar.activation(yv[:], impv[:], AF.Exp)
                nc.vector.tensor_scalar(out=yv[:], in0=yv[:], scalar1=1.0,
                                        scalar2=None, op0=OP.add)
                sq_ = T(rp, [8, 8], F32, "r_sps", f"sps_{l}")
                nc.scalar.activation(sq_[:], yv[:], AF.Sqrt)
                nc.vector.reciprocal(sq_[:], sq_[:])
                wv = T(rp, [8, 8], F32, "r_spw", f"spw_{l}")
                nc.vector.tensor_scalar(out=wv[:], in0=yv[:], scalar1=-1.0,
                                        scalar2=None, op0=OP.add)
                nc.vector.tensor_tensor(out=wv[:], in0=wv[:], in1=sq_[:], op=OP.mult)
                ev = T(rp, [8, 8], F32, "r_spe", f"spe_{l}")
                for it_ in range(3):
                    nc.scalar.activation(ev[:], wv[:], AF.Exp, scale=-1.0)
                    nc.vector.tensor_tensor(out=ev[:], in0=yv[:], in1=ev[:], op=OP.mult)
                    nc.vector.tensor_tensor(out=wv[:], in0=wv[:], in1=ev[:], op=OP.add)
                    nc.vector.tensor_scalar(out=wv[:], in0=wv[:], scalar1=-1.0,
                                            scalar2=None, op0=OP.add)
                nc.vector.tensor_copy(out=impv[:], in_=wv[:])
                nc.vector.tensor_tensor(out=impv[:], in0=impv[:], in1=eyec_s[:], op=OP.mult)
                nc.vector.tensor_scalar(out=impv[:], in0=impv[:], scalar1=1.0,
                                        scalar2=None, op0=OP.add)
                nc.vector.reciprocal(impv[:], impv[:])
                if dbg and l == 0:
                    nc.sync.dma_start(out=dbgT[0:8, 12400:12408], in_=impv[:])
                coef = T(rp, [8, 8], F32, "r_coef", f"coef_{l}")
                nc.vector.tensor_tensor(out=coef[:], in0=impv[:], in1=pre8_s[:], op=OP.mult)
                if dbg and l == 0:
                    nc.sync.dma_start(out=dbgT[0:8, 12392:12400], in_=coef[:])
                nc.gpsimd.memset(c32[:], 0.0)
                nc.vector.tensor_copy(out=c32[0:8, 0:8], in_=coef[:])
                nc.vector.transpose(ct32[:], c32[:])
                ctb = T(rp, [8, 8], BF16, "r_ctb", f"ctb_{l}")
                nc.scalar.copy(out=ctb[:], in_=ct32[0:8, 0:8])
                c768 = T(ap_, [128, KT*8], F32, "c768", f"c768_{l}")
                for mi in range(KT):
                    pe = pp.tile([128, 8], F32, tag="pr0", name=f"c768_{l}_{mi}")
                    nc.tensor.matmul(pe[:], e8bf_s[:, mi*128:(mi+1)*128], ctb[:],
                                     start=True, stop=True)
                    nc.scalar.copy(out=c768[:, mi*8:(mi+1)*8], in_=pe[:])
                if dbg and l == 0:
                    nc.sync.dma_start(out=dbgT[:, 12288:12336], in_=c768[:])
                merged_bf = T(ap_, [128, KT*S], BF16, "merged", f"merged_{l}")
                # head-ranges per output tile b: (n, c0, c1, h0)
                for b in range(KT):
                    mwb = T(ap_, [128, KT*128], BF16, f"mwB{b % 2}", f"mwb_{l}_{b}")
                    n0, n1 = (128*b)//96, (128*b+127)//96
                    for a in range(KT):
                        for n in range(n0, n1+1):
                            c0 = max(0, n*96 - 128*b)
                            c1 = min(128, (n+1)*96 - 128*b)
                            h0 = 128*b + c0 - n*96
                            eng = nc.vector if (a + n) % 2 == 0 else nc.gpsimd
                            eng.tensor_scalar(
                                out=mwb[:, a*128+c0:a*128+c1],
                                in0=ist_s[:, a*96+h0:a*96+h0+(c1-c0)],
                                scalar1=c768[:, a*8+n:a*8+n+1],
                                scalar2=None, op0=OP.mult)
                    pT = pp.tile([128, S], F32, tag=f"pm{b % 4}", name=f"mix_{l}_{b}")
                    for a in range(KT):
                        nc.tensor.matmul(pT[:], mwb[:, a*128:(a+1)*128],
                                         hf_bf[:, a*S:(a+1)*S],
                                         start=(a == 0), stop=(a == KT-1))
                    nc.vector.tensor_tensor(out=merged_bf[:, b*S:(b+1)*S],
                                            in0=heads_bf[:, b*S:(b+1)*S],
                                            in1=pT[:], op=OP.add)
                if dbg and l == 0:
                    nc.sync.dma_start(out=dbgB[:, 3072:4608], in_=merged_bf[:])
                pre = T(ap_, [128, KT*S], F32, "pre", f"pre_{l}")
                for mi in range(KT):
                    ps = pp.tile([128, S], F32, tag=f"pm{mi}", name=f"outw_{l}_{mi}")
                    mm_block(ps[:], out_w, KT, mi, merged_bf, range(KT))
                    nc.vector.tensor_scalar(out=pre[:, mi*S:(mi+1)*S], in0=ps[:],
                                            scalar1=bias[:, OUT_B+mi:OUT_B+mi+1],
                                            scalar2=None, op0=OP.add)
                    nc.vector.tensor_tensor(out=pre[:, mi*S:(mi+1)*S],
                                            in0=pre[:, mi*S:(mi+1)*S],
                                            in1=x[:, mi*S:(mi+1)*S], op=OP.add)
                if dbg and l == 0:
                    nc.sync.dma_start(out=dbgT[:, 1536:3072], in_=pre[:])
                oh = T(ap_, [128, KT*S], F32, "oh", f"oh_{l}")
                layernorm(f"ln1_{l}", pre, bias[:, LN1G:LN1G+6], bias[:, LN1B:LN1B+6], oh)
                oh_bf = T(ap_, [128, KT*S], BF16, "ohbf", f"ohbf_{l}")
                for ki in range(KT):
                    nc.scalar.copy(out=oh_bf[:, ki*S:(ki+1)*S], in_=oh[:, ki*S:(ki+1)*S])
                if dbg and l == 0:
                    nc.sync.dma_start(out=dbgT[:, 3072:4608], in_=oh[:])
                pg8 = pp.tile([8, S], F32, tag="pr0", name=f"g8_{l}")
                for ki in range(KT):
                    nc.tensor.matmul(pg8[:], igw_s[:, ki*8:(ki+1)*8],
                                     oh_bf[:, ki*S:(ki+1)*S],
                                     start=(ki == 0), stop=(ki == KT-1))
                g8b = T(rp, [8, S], BF16, "r_g8b", f"g8b_{l}")
                nc.scalar.activation(g8b[:], pg8[:], AF.Sigmoid,
                                     bias=sbias_s[0:8, IGB:IGB+1])
                jz = T(ap_, [128, KT*S], F32, "jz", f"jz_{l}")
                jz_bf = T(ap_, [128, KT*S], BF16, "jzbf", f"jzbf_{l}")
                for mi in range(KT):
                    phe = pp.tile([128, S], F32, tag=f"pm{mi}", name=f"he_{l}_{mi}")
                    for (ki, si) in meta['zone_map'][mi]:
                        nc.tensor.matmul(phe[:], zone_s[:, si*128:(si+1)*128],
                                         oh_bf[:, ki*S:(ki+1)*S],
                                         start=(si == meta['zone_map'][mi][0][1]),
                                         stop=False)
                    nc.tensor.matmul(phe[:], zw2_s[:, mi*128:(mi+1)*128], pc16_s[:],
                                     start=False, stop=True)
                    pgx = pp.tile([128, S], F32, tag="pr1", name=f"gx_{l}_{mi}")
                    nc.tensor.matmul(pgx[:], e8bf_s[:, mi*128:(mi+1)*128], g8b[:],
                                     start=True, stop=True)
                    t1 = T(ap_, [128, S], F32, "zt1", f"zt1_{l}_{mi}", bufs=2)
                    nc.vector.tensor_tensor(out=t1[:], in0=phe[:],
                                            in1=oh[:, mi*S:(mi+1)*S], op=OP.subtract)
                    t2 = T(ap_, [128, S], F32, "zt2", f"zt2_{l}_{mi}", bufs=2)
                    nc.vector.tensor_tensor(out=t2[:], in0=t1[:], in1=pgx[:], op=OP.mult)
                    nc.vector.tensor_tensor(out=jz[:, mi*S:(mi+1)*S],
                                            in0=oh[:, mi*S:(mi+1)*S], in1=t2[:], op=OP.add)
                    nc.scalar.copy(out=jz_bf[:, mi*S:(mi+1)*S], in_=jz[:, mi*S:(mi+1)*S])
                if dbg and l == 0:
                    nc.sync.dma_start(out=dbgT[:, 4608:6144], in_=jz[:])
                s1b = T(ap_, [128, 3*S], BF16, "s1b", f"s1b_{l}")
                for mi in range(3):
                    ps1 = pp.tile([128, S], F32, tag=f"pm{mi}", name=f"s1_{l}_{mi}")
                    sl = meta['sh1_map'][mi]
                    for j, (ki, si) in enumerate(sl):
                        nc.tensor.matmul(ps1[:], sh1_w[:, si*128:(si+1)*128],
                                         jz_bf[:, ki*S:(ki+1)*S],
                                         start=(j == 0), stop=(j == len(sl)-1))
                    nc.scalar.activation(s1b[:, mi*S:(mi+1)*S], ps1[:], AF.Gelu,
                                         bias=bias[:, SH1B+mi:SH1B+mi+1])
                pld = pp.tile([8, S], F32, tag="pr0", name=f"ld_{l}")
                for ki in range(3):
                    nc.tensor.matmul(pld[:], sh2_w[:, ki*8:(ki+1)*8],
                                     s1b[:, ki*S:(ki+1)*S],
                                     start=(ki == 0), stop=(ki == 2))
                sig_d = T(rp, [8, S], BF16, "r_sigd", f"sig_{l}")
                lds = T(rp, [8, 1], F32, "r_lds", f"lds_{l}")
                nc.scalar.activation(sig_d[:], pld[:], AF.Sigmoid,
                                     bias=bias[0:8, SH2B:SH2B+1], accum_out=lds[:])
                absc = T(ap_, [128, KT], F32, "absc", f"absc_{l}")
                for ki in range(KT):
                    d6 = T(ap_, [128, S], F32, "zt1", f"d6_{l}_{ki}", bufs=2)
                    nc.vector.tensor_tensor(out=d6[:], in0=jz[:, ki*S:(ki+1)*S],
                                            in1=x[:, ki*S:(ki+1)*S], op=OP.subtract)
                    nc.vector.tensor_reduce(absc[:, ki:ki+1], d6[:],
                                            axis=mybir.AxisListType.X, op=OP.add,
                                            apply_absolute_value=True)
                pcd = pp.tile([8, 1], F32, tag="pr1", name=f"cd_{l}")
                for ki in range(KT):
                    nc.tensor.matmul(pcd[:], e8c_s[:, ki*8:(ki+1)*8], absc[:, ki:ki+1],
                                     start=(ki == 0), stop=(ki == KT-1))
                heat(8)
                cds = T(rp, [8, 1], F32, "r_cds", f"cds_{l}")
                nc.vector.tensor_copy(out=cds[:], in_=pcd[:])
                nc.sync.dma_start(out=arin[l][0:8, :], in_=lds[:])
                nc.sync.dma_start(out=arin[l][32:40, :], in_=cds[:])
                nc.gpsimd.collective_compute(
                    "AllReduce", OP.add, ins=[arin[l][:]], outs=[arout[l][:]],
                    replica_groups=[list(range(N_CORES))])
                arb = T(rp, [64, 1], F32, "r_arb", f"arb_{l}")
                nc.sync.dma_start(out=arb[:], in_=arout[l][:])
                if dbg and l == 0:
                    nc.sync.dma_start(out=dbgT[0:8, 12342:12343], in_=lds[:])
                    nc.sync.dma_start(out=dbgT[0:8, 12343:12344], in_=cds[:])
                    nc.sync.dma_start(out=dbgT[0:64, 12344:12345], in_=arb[:])
                ldm = T(rp, [8, 1], F32, "r_ldm", f"ldm_{l}")
                nc.scalar.activation(ldm[:], arb[0:8, :], AF.Copy, scale=1.0/(8*S))
                sgd = T(rp, [8, 1], F32, "r_sgd", f"sgd_{l}")
                nc.scalar.activation(sgd[:], arb[32:40, :], AF.Sigmoid,
                                     scale=10.0/(8*S*HD),
                                     bias=sbias_s[0:8, NEG1:NEG1+1])
                nc.vector.tensor_tensor(out=ldm[:], in0=ldm[:], in1=sgd[:], op=OP.mult)
                mask8 = T(rp, [8, 1], F32, "r_msk", f"mask8_{l}")
                nc.vector.tensor_scalar(out=mask8[:], in0=ldm[:], scalar1=THR,
                                        scalar2=None, op0=OP.is_ge)
                mask8b = T(rp, [8, 1], BF16, "r_mskb", f"mask8b_{l}")
                nc.scalar.copy(out=mask8b[:], in_=mask8[:])
                m768 = T(ap_, [128, KT], F32, "m768", f"m768_{l}")
                for mi in range(KT):
                    pmx = pp.tile([128, 1], F32, tag="pr0", name=f"mx_{l}_{mi}")
                    nc.tensor.matmul(pmx[:], e8bf_s[:, mi*128:(mi+1)*128], mask8b[:],
                                     start=True, stop=True)
                    nc.scalar.copy(out=m768[:, mi:mi+1], in_=pmx[:])
                if dbg and l == 0:
                    nc.sync.dma_start(out=dbgT[:, 12336:12342], in_=m768[:])
                jzm = T(ap_, [128, KT*S], F32, "pre", f"jzm_{l}")
                jzm_bf = T(ap_, [128, KT*S], BF16, "jzbf2", f"jzmbf_{l}")
                for ki in range(KT):
                    nc.vector.tensor_scalar(out=jzm[:, ki*S:(ki+1)*S],
                                            in0=jz[:, ki*S:(ki+1)*S],
                                            scalar1=m768[:, ki:ki+1], scalar2=None,
                                            op0=OP.mult)
                    nc.vector.tensor_scalar(out=jzm_bf[:, ki*S:(ki+1)*S],
                                            in0=jz[:, ki*S:(ki+1)*S],
                                            scalar1=m768[:, ki:ki+1], scalar2=None,
                                            op0=OP.mult)
                if dbg and l == 0:
                    nc.sync.dma_start(out=dbgT[:, 6144:7680], in_=jzm[:])
                sm = T(ap_, [128, KT*S], F32, "smx", f"sm_{l}")
                for mi in range(KT):
                    pcg = pp.tile([128, S], F32, tag=f"pm{mi % 3}", name=f"cg_{l}_{mi}")
                    mm_block(pcg[:], cgw_s, KT, mi, x_bf, range(KT))
                    cgf = T(ap_, [128, S], F32, "cgf", f"cgf_{l}_{mi}", bufs=2)
                    nc.scalar.activation(cgf[:], pcg[:], AF.Sigmoid,
                                         bias=sbias_s[:, CGB+mi:CGB+mi+1])
                    pdg = pp.tile([128, S], F32, tag=f"pm{3 + mi % 3}", name=f"dg_{l}_{mi}")
                    mm_block(pdg[:], dgw_s, KT, mi, jzm_bf, range(KT))
                    dgf = T(ap_, [128, S], F32, "dgf", f"dgf_{l}_{mi}", bufs=2)
                    nc.scalar.activation(dgf[:], pdg[:], AF.Sigmoid,
                                         bias=sbias_s[:, DGB+mi:DGB+mi+1])
                    t1 = T(ap_, [128, S], F32, "zt1", f"bt1_{l}_{mi}", bufs=2)
                    nc.vector.tensor_tensor(out=t1[:], in0=cgf[:],
                                            in1=x[:, mi*S:(mi+1)*S], op=OP.mult)
                    nc.vector.tensor_tensor(out=t1[:], in0=t1[:], in1=dgf[:], op=OP.mult)
                    nc.vector.tensor_tensor(out=sm[:, mi*S:(mi+1)*S],
                                            in0=jzm[:, mi*S:(mi+1)*S], in1=t1[:],
                                            op=OP.add)
                if dbg and l == 0:
                    nc.sync.dma_start(out=dbgT[:, 7680:9216], in_=sm[:])
                f_bf = T(ap_, [128, KT*S], BF16, "fbf", f"fbf_{l}")
                layernorm(f"ffln_{l}", sm, bias[:, FFLNG:FFLNG+6],
                          bias[:, FFLNB:FFLNB+6], f_bf)
                if dbg and l == 0:
                    nc.sync.dma_start(out=dbgB[:, 4608:6144], in_=f_bf[:])
                heat(3)
                pf2s = [pp.tile([128, S], F32, tag=f"pm{mi}", name=f"ff2_{l}_{mi}")
                        for mi in range(KT)]
                for half in range(2):
                    ff1_w = T(wp, [128, 72*128], BF16, "ff1w", f"ff1w_{l}_{half}")
                    nc.sync.dma_start(
                        out=ff1_w[:],
                        in_=ff1w[:, (l*144+half*72)*128:(l*144+(half+1)*72)*128])
                    ff2_w = T(wp, [128, 72*128], BF16, "ff2w", f"ff2w_{l}_{half}")
                    nc.sync.dma_start(
                        out=ff2_w[:],
                        in_=ff2w[:, (l*144+half*72)*128:(l*144+(half+1)*72)*128])
                    h_bf = T(ap_, [128, 12*S], BF16, "hbf", f"hbf_{l}_{half}")
                    for mj in range(12):
                        mi = half*12 + mj
                        ph = pp.tile([128, S], F32, tag=f"pr{mj % 2}",
                                     name=f"ff1_{l}_{mi}")
                        # ff1 half blob: tiles (ki, mi_local=mi-half*12), nm=12
                        for ki in range(KT):
                            nc.tensor.matmul(ph[:],
                                             ff1_w[:, (ki*12+mj)*128:(ki*12+mj+1)*128],
                                             f_bf[:, ki*S:(ki+1)*S],
                                             start=(ki == 0), stop=(ki == KT-1))
                        nc.scalar.activation(h_bf[:, mj*S:(mj+1)*S], ph[:], AF.Gelu,
                                             bias=bias[:, FFB1+mi:FFB1+mi+1])
                    for mi in range(KT):
                        for kj in range(12):
                            # ff2 half blob: tiles (ki_local=kj, mi), nm=6
                            nc.tensor.matmul(pf2s[mi][:],
                                             ff2_w[:, (kj*KT+mi)*128:(kj*KT+mi+1)*128],
                                             h_bf[:, kj*S:(kj+1)*S],
                                             start=(half == 0 and kj == 0),
                                             stop=(half == 1 and kj == 11))
                x_n = T(ap_, [128, KT*S], F32, "x0", f"x_l{l+1}")
                xbf_n = T(ap_, [128, KT*S], BF16, "xbf0", f"xbf_l{l+1}")
                for mi in range(KT):
                    t1 = T(ap_, [128, S], F32, "zt2", f"xo_{l}_{mi}", bufs=2)
                    nc.vector.tensor_scalar(out=t1[:], in0=pf2s[mi][:],
                                            scalar1=bias[:, FFB2+mi:FFB2+mi+1],
                                            scalar2=None, op0=OP.add)
                    nc.vector.tensor_tensor(out=x_n[:, mi*S:(mi+1)*S],
                                            in0=sm[:, mi*S:(mi+1)*S], in1=t1[:], op=OP.add)
                    nc.scalar.copy(out=xbf_n[:, mi*S:(mi+1)*S], in_=x_n[:, mi*S:(mi+1)*S])
                heat(4)
                if dbg and l == 0:
                    nc.sync.dma_start(out=dbgT[:, 9216:10752], in_=x_n[:])
                x, x_bf = x_n, xbf_n

            # ================= memory scan =================
            zr_s = ld(zrww, [128, KT*32], BF16, "zrwS", "zrw_s")
            zw_s = ld(zwww, [128, KT*32], BF16, "zwwS", "zww_s")
            zm_s = ld(zmtw, [128, KT*32], BF16, "zmtS", "zmt_s")
            zmp_s = ld(zmpw, [128, 6*128], BF16, "zmpS", "zmp_s")
            zog_s = T(wp, [128, 72*128], BF16, "ff1w", "zog_s")
            nc.sync.dma_start(out=zog_s[:], in_=zogw[:])

            def small_mm(blob, act, name, ptag):
                psx = pp.tile([32, S], F32, tag=ptag, name=name)
                for ki in range(KT):
                    nc.tensor.matmul(psx[:], blob[:, ki*32:(ki+1)*32],
                                     act[:, ki*S:(ki+1)*S],
                                     start=(ki == 0), stop=(ki == KT-1))
                return psx

            prw = small_mm(zr_s, x_bf, "prw", "pr0")
            rw = T(rp, [32, S], F32, "r_rw", "rw_s")
            nc.scalar.activation(rw[:], prw[:], AF.Sigmoid,
                                 bias=sbias_s[0:32, ZRWB:ZRWB+1])
            pww = small_mm(zw_s, x_bf, "pww", "pr1")
            ww = T(rp, [32, S], F32, "r_ww", "ww_s")
            nc.scalar.activation(ww[:], pww[:], AF.Sigmoid,
                                 bias=sbias_s[0:32, ZWWB:ZWWB+1])
            pnm = small_mm(zm_s, x_bf, "pnm", "pr0")
            nmt = T(rp, [32, S], F32, "r_nmt", "nmt_s")
            nc.vector.tensor_scalar(out=nmt[:], in0=pnm[:],
                                    scalar1=sbias_s[0:32, ZMTB:ZMTB+1], scalar2=None,
                                    op0=OP.add)
            af = T(rp, [32, S], F32, "r_af", "af_s")
            nc.vector.tensor_scalar(out=af[:], in0=ww[:], scalar1=-1.0, scalar2=1.0,
                                    op0=OP.mult, op1=OP.add)
            bf_ = T(rp, [32, S], F32, "r_bf", "bf_s")
            nc.vector.tensor_tensor(out=bf_[:], in0=ww[:], in1=nmt[:], op=OP.mult)
            Msc = T(rp, [32, S], F32, "r_M", "M_s")
            nc.vector.tensor_tensor_scan(Msc[:], af[:], bf_[:], 0.0, OP.mult, OP.add)
            Mp = T(rp, [32, S], F32, "r_nmt", "Mp_s")
            nc.gpsimd.memset(Mp[:, 0:1], 0.0)
            nc.vector.tensor_copy(out=Mp[:, 1:S], in_=Msc[:, 0:S-1])
            rwm = T(rp, [32, S], F32, "r_af", "rwm_s")
            nc.vector.tensor_tensor(out=rwm[:], in0=rw[:], in1=Mp[:], op=OP.mult)
            rwm_b = T(rp, [32, S], BF16, "r_rwmb", "rwmb_s")
            nc.scalar.copy(out=rwm_b[:], in_=rwm[:])
            mv_bf = T(ap_, [128, KT*S], BF16, "lnb", "mvbf_s")
            for mi in range(KT):
                pmv = pp.tile([128, S], F32, tag=f"pm{mi}", name=f"mv_{mi}")
                nc.tensor.matmul(pmv[:], zmp_s[0:32, mi*128:(mi+1)*128], rwm_b[:],
                                 start=True, stop=True)
                t1 = T(ap_, [128, S], F32, "zt1", f"mvt_{mi}", bufs=2)
                nc.vector.tensor_scalar(out=t1[:], in0=pmv[:],
                                        scalar1=sbias_s[:, ZMPB+mi:ZMPB+mi+1],
                                        scalar2=None, op0=OP.add)
                nc.scalar.copy(out=mv_bf[:, mi*S:(mi+1)*S], in_=t1[:])
            fused = T(ap_, [128, KT*S], F32, "pre", "fused_s")
            for mi in range(KT):
                pfu = pp.tile([128, S], F32, tag=f"pm{mi}", name=f"fu_{mi}")
                for ki in range(12):
                    rhs = x_bf[:, ki*S:(ki+1)*S] if ki < KT else \
                        mv_bf[:, (ki-KT)*S:(ki-KT+1)*S]
                    nc.tensor.matmul(pfu[:], zog_s[:, (ki*KT+mi)*128:(ki*KT+mi+1)*128],
                                     rhs, start=(ki == 0), stop=(ki == 11))
                nc.scalar.activation(fused[:, mi*S:(mi+1)*S], pfu[:], AF.Tanh,
                                     bias=sbias_s[:, ZOGB+mi:ZOGB+mi+1])
            if dbg:
                nc.sync.dma_start(out=dbgT[:, 10752:12288], in_=fused[:])
            y_bf = T(ap_, [128, KT*S], BF16, "jzbf", "ybf_s")
            layernorm("hln", fused, sbias_s[:, HLNG:HLNG+6], sbias_s[:, HLNB:HLNB+6],
                      y_bf)

            if dbg:
                nc.sync.dma_start(out=dbgB[:, 9216:10752], in_=y_bf[:])
            # ================= head =================
            for mi in range(head_mt):
                hwt = T(wp, [128, KT*128], BF16, f"hw{mi % 2}", f"hw_{mi}")
                for ki in range(KT):
                    nc.sync.dma_start(
                        out=hwt[:, ki*128:(ki+1)*128],
                        in_=headw[:, (ki*HEADMT+mi)*128:(ki*HEADMT+mi+1)*128])
                ph = pp.tile([128, S], F32, tag=f"pm{mi % 6}", name=f"hd_{mi}")
                for ki in range(KT):
                    nc.tensor.matmul(ph[:], hwt[:, ki*128:(ki+1)*128],
                                     y_bf[:, ki*S:(ki+1)*S],
                                     start=(ki == 0), stop=(ki == KT-1))
                ob = T(ap_, [128, S], F32, f"ob{mi % 3}", f"ob_{mi}")
                nc.scalar.copy(out=ob[:], in_=ph[:])
                nc.sync.dma_start(out=out[:, mi*S:(mi+1)*S], in_=ob[:])

    nc.compile()
    return nc


_CACHE = {}


def _get_nc(n_layers, head_mt, meta, dbg=False):
    key = (n_layers, head_mt, dbg)
    if key not in _CACHE:
        _CACHE[key] = build(n_layers, head_mt, meta, dbg=dbg)
    return _CACHE[key]


def kernel(input_ids, params, n_layers=LFULL, head_mt=HEADMT, want_trace=False, dbg=False):
    from concourse.bass_utils import run_bass_kernel_spmd
    g, meta = prep_host(params, n_layers)
    nc = _get_nc(n_layers, head_mt, meta, dbg=dbg)
    ids = np.asarray(input_ids)
    in_maps = []
    for c in range(N_CORES):
        m = dict(g)
        m['ids'] = np.ascontiguousarray(
            ids[c].astype(np.int32).reshape(2, 128).T)
        in_maps.append(m)
    trace = False
    if want_trace:
        try:
            import ntff_shim
            ntff_shim.install()
            trace = True
        except Exception:
            pass
    if trace:
        # axon NTFF profiling needs an initialized PJRT client: warm run first
        run_bass_kernel_spmd(nc, in_maps, list(range(N_CORES)), trace=False)
        try:
            res = run_bass_kernel_spmd(nc, in_maps, list(range(N_CORES)), trace=True)
        except Exception as e:
            print("trace failed:", e)
            res = run_bass_kernel_spmd(nc, in_maps, list(range(N_CORES)), trace=False)
    else:
        res = run_bass_kernel_spmd(nc, in_maps, list(range(N_CORES)), trace=False)
    outs = []
    for c in range(N_CORES):
        blob = res.results[c]["out"]
        outs.append(blob.reshape(128, head_mt, S).transpose(2, 1, 0)
                    .reshape(S, head_mt*128))
    logits = np.stack(outs, 0).astype(np.float32)
    kernel.last_exec_ns = res.exec_time_ns
    if dbg:
        kernel.dbg = res.results[0].get("dbg")
        kernel.dbgb = res.results[0].get("dbgb")
    return logits
